# revision 20
# baseline (speedup 1.0000x reference)
"""Trainium2 Bass kernel for nn_CrossAttention (B=4, Nq=Nk=2048, D=1024, H=16).

Sharding: 8 cores = (batch b in 0..3) x (head-group hg in 0..1), 8 heads/core.
Each core gets its batch's query/context plus the column slice of Wq/Wk/Wv and
row slice of Wo for its 8 heads; LayerNorm params are replicated.  Host sums
the two head-group partial outputs per batch and adds bo.

Per-core pipeline (all matmuls bf16 with fp32 PSUM accumulation):
  Phase 1 (context): LN (fp32, bn_stats; rstd = exp(-0.5*ln(var+eps)) so the
  whole kernel uses ONE activation table set - no table-switch stalls)
  -> PE transpose -> K^T / V projections.  PSUM->SBUF moves ride ScalarE /
  DVE (ScalarE is otherwise idle here).
  Phase 2 (query proj + attention, software-pipelined): the q-chunk LN /
  transpose / Q^T projection for chunk c+1 is emitted inside the attention
  loop over chunk c, so DVE/PE/Pool chew projection work while ScalarE
  streams exp.  ScalarE does NOTHING but exp in this phase (the Q^T
  PSUM->SBUF moves ride the Pool engine).
  S^T = K Q^T tiles (2 heads packed in the 128-row PE array via auto
  tile_position row tiling -> concurrent matmul pairs on HW)
  -> exp on ScalarE with the 1/sqrt(dh) scale folded in
  -> AV matmul with M=65 (row 64 = softmax denominator Z, for free)
  -> normalize via reciprocal_approx_fast straight from PSUM + GPSIMD
  partition_broadcast + DVE -> Wo row-slice matmul (deferred half an
  iteration so the PE queue head never blocks on the normalize chain)
  -> fp32 partial output.
"""

import numpy as np

import concourse.bass as bass
import concourse.mybir as mybir
import concourse.tile as tile
from concourse import bacc
from concourse.masks import make_identity

P = 128
N_TOK = 2048          # tokens per batch (both Nq and Nk)
D = 1024              # model dim
KS = D // P           # 8 contraction subtiles
DG = 512              # per-core projection width (8 heads * 64)
NM = DG // P          # 4 output blocks / head-pair groups
NH = 8                # heads per core
HD = 64
NT = N_TOK // P       # 16 token tiles
NCH = N_TOK // 512    # 4 token chunks of 512
SCALE = HD ** -0.5
EPS = 1e-5

F32 = mybir.dt.float32
BF16 = mybir.dt.bfloat16
_UNIQ = [0]


def _build_program(ln_affine=True, with_bias=True, repeat=1, hw_loop=0,
                   probe="full"):
    nc = bacc.Bacc("TRN2", target_bir_lowering=False, debug=False)

    q_in = nc.dram_tensor("q_in", (N_TOK, D), F32, kind="ExternalInput")
    c_in = nc.dram_tensor("c_in", (N_TOK, D), F32, kind="ExternalInput")
    wq = nc.dram_tensor("wq", (D, DG), BF16, kind="ExternalInput")
    wk = nc.dram_tensor("wk", (D, DG), BF16, kind="ExternalInput")
    wv = nc.dram_tensor("wv", (D, DG), BF16, kind="ExternalInput")
    wo = nc.dram_tensor("wo", (DG, D), BF16, kind="ExternalInput")
    if with_bias:
        bq_d = nc.dram_tensor("bq", (DG,), F32, kind="ExternalInput")
        bk_d = nc.dram_tensor("bk", (DG,), F32, kind="ExternalInput")
        bv_d = nc.dram_tensor("bv", (DG,), F32, kind="ExternalInput")
    else:
        bq_d = bk_d = bv_d = None
    if ln_affine:
        gq_d = nc.dram_tensor("gq", (D,), F32, kind="ExternalInput")
        btq_d = nc.dram_tensor("btq", (D,), F32, kind="ExternalInput")
        gkv_d = nc.dram_tensor("gkv", (D,), F32, kind="ExternalInput")
        btkv_d = nc.dram_tensor("btkv", (D,), F32, kind="ExternalInput")
    else:
        gq_d = btq_d = gkv_d = btkv_d = None
    y_out = nc.dram_tensor("y_out", (N_TOK, D), F32, kind="ExternalOutput")

    import contextlib

    with tile.TileContext(nc) as tc:
        loop_ctx = tc.For_i(0, hw_loop, 1) if hw_loop else None
        with (loop_ctx if loop_ctx is not None else contextlib.nullcontext()):
         for _rep in range(repeat):
            _UNIQ[0] += 1
            _emit_kernel(nc, tc, q_in, c_in, wq, wk, wv, wo,
                         bq_d, bk_d, bv_d, gq_d, btq_d, gkv_d, btkv_d,
                         y_out, ln_affine, with_bias, probe)

    nc.finalize()
    return nc


def _emit_kernel(nc, tc, q_in, c_in, wq, wk, wv, wo,
                 bq_d, bk_d, bv_d, gq_d, btq_d, gkv_d, btkv_d,
                 y_out, ln_affine, with_bias, probe="full"):
    # probe: "ctx" (phase 1 + qproj only), "sexp" (+ S/exp), "av" (+ AV),
    #        "norm" (+ normalize), "full"
    _LV = {"ctx": 0, "sexp": 1, "av": 2, "norm": 3, "full": 4}[probe]
    uq = _UNIQ[0]
    exp_bufs = 1 if ln_affine else 3
    with (
        tc.tile_pool(name="persist", bufs=1) as persist,
        tc.tile_pool(name="wqo", bufs=1) as wqo,
        tc.tile_pool(name="consts", bufs=1) as consts,
        tc.tile_pool(name="stats", bufs=4) as stats,
    ):
        # ---------------- persistent tensors ----------------
        qt = [persist.tile([P, NM, 512], BF16, tag=f"qt{c}", name=f"qt{c}_{uq}")
              for c in range(NCH)]   # Q^T per token chunk
        kt = [persist.tile([P, NM, 512], BF16, tag=f"kt{c}", name=f"kt{c}_{uq}")
              for c in range(NCH)]   # K^T per key chunk
        vs = persist.tile([P, NT, NH, HD + 1], BF16, tag="vs")
        os_t = [persist.tile([P, NM, 512], BF16, tag=f"os{c}",
                             name=f"os{c}_{uq}")
                for c in range(NCH)]
        nc.vector.memset(vs[:, :, :, HD:HD + 1], 1.0)
        wo_bf = wqo.tile([P, NM, D], BF16, tag="wo_bf")

        # ---------------- constants ----------------
        ident = consts.tile([P, P], BF16, tag="ident")
        make_identity(nc, ident)
        eps_t = consts.tile([P, 1], F32, tag="eps")
        nc.vector.memset(eps_t, EPS)
        if ln_affine:
            gq_b = consts.tile([P, D], F32, tag="gq_b")
            nc.gpsimd.dma_start(out=gq_b,
                                in_=gq_d[None, :].to_broadcast((P, D)))
            btq_b = consts.tile([P, D], F32, tag="btq_b")
            nc.gpsimd.dma_start(out=btq_b,
                                in_=btq_d[None, :].to_broadcast((P, D)))
            gkv_b = consts.tile([P, D], F32, tag="gkv_b")
            nc.gpsimd.dma_start(out=gkv_b,
                                in_=gkv_d[None, :].to_broadcast((P, D)))
            btkv_b = consts.tile([P, D], F32, tag="btkv_b")
            nc.gpsimd.dma_start(out=btkv_b,
                                in_=btkv_d[None, :].to_broadcast((P, D)))
        else:
            gq_b = btq_b = gkv_b = btkv_b = None
        if with_bias:
            bv_b = consts.tile([P, DG], F32, tag="bv_b")
            nc.gpsimd.dma_start(out=bv_b,
                                in_=bv_d[None, :].to_broadcast((P, DG)))
            bq_c = consts.tile([P, NM], F32, tag="bq_c")
            nc.sync.dma_start(out=bq_c,
                              in_=bq_d.rearrange("(m p) -> p m", p=P))
            bk_c = consts.tile([P, NM], F32, tag="bk_c")
            nc.sync.dma_start(out=bk_c,
                              in_=bk_d.rearrange("(m p) -> p m", p=P))
        else:
            bv_b = bq_c = bk_c = None

        nc.gpsimd.dma_start(out=wo_bf,
                            in_=wo.rearrange("(m p) n -> p m n", p=P))

        c_r = c_in.rearrange("(n i p) d -> n p i d", p=P, i=4)
        q_r = q_in.rearrange("(n i p) d -> n p i d", p=P, i=4)

        lncnt = [0]

        def ln_chunk(xb, g_b, b_b, lnpool, lntmp=None):
            """LN 4 token tiles xb[:, tl, :] -> list of [128, 1024] bf16."""
            lncnt[0] += 1
            mv = stats.tile([P, 4, 2], F32, tag="mv")
            for tl in range(4):
                st = stats.tile([P, 2, 6], F32, tag="bnst")
                nc.vector.bn_stats(out=st[:, 0, :], in_=xb[:, tl, 0:512])
                nc.vector.bn_stats(out=st[:, 1, :], in_=xb[:, tl, 512:1024])
                nc.vector.bn_aggr(out=mv[:, tl, :], in_=st)
            # rstd = exp(-0.5 * ln(var + eps)) -- stays in the exp table set
            lnv = stats.tile([P, 4], F32, tag="lnv")
            nc.scalar.activation(out=lnv, in_=mv[:, :, 1],
                                 func=mybir.ActivationFunctionType.Ln,
                                 bias=eps_t)
            rstd = stats.tile([P, 4], F32, tag="rstd")
            nc.scalar.activation(out=rstd, in_=lnv,
                                 func=mybir.ActivationFunctionType.Exp,
                                 scale=-0.5)
            lnts = []
            for tl in range(4):
                x = xb[:, tl, :]
                lnt = lnpool.tile([P, D], BF16, tag="ln",
                                  name=f"ln{tl}_{uq}_{lncnt[0]}")
                if not ln_affine:
                    nc.vector.tensor_scalar(
                        out=lnt, in0=x, scalar1=mv[:, tl, 0:1],
                        scalar2=rstd[:, tl:tl + 1],
                        op0=mybir.AluOpType.subtract,
                        op1=mybir.AluOpType.mult)
                else:
                    xc = lntmp.tile([P, D], F32, tag="xc")
                    nc.vector.tensor_scalar(
                        out=xc, in0=x, scalar1=mv[:, tl, 0:1],
                        scalar2=rstd[:, tl:tl + 1],
                        op0=mybir.AluOpType.subtract,
                        op1=mybir.AluOpType.mult)
                    xg = lntmp.tile([P, D], F32, tag="xg")
                    nc.vector.tensor_tensor(out=xg, in0=xc, in1=g_b,
                                            op=mybir.AluOpType.mult)
                    nc.vector.tensor_tensor(out=lnt, in0=xg, in1=b_b,
                                            op=mybir.AluOpType.add)
                lnts.append(lnt)
            return lnts

        def transpose_chunk(ln_tiles, ps_pool, lnT):
            """4 LN tiles ([128 tok, 1024 feat]) -> lnT [128 feat, 8, 512 tok]."""
            for s in range(KS):
                pt = ps_pool.tile([P, 512], BF16, tag="tr")
                for tl in range(4):
                    nc.tensor.transpose(pt[:, tl * P:(tl + 1) * P],
                                        ln_tiles[tl][:, s * P:(s + 1) * P],
                                        ident)
                nc.scalar.copy(out=lnT[:, s, :], in_=pt)
            return lnT

        # ========= phase 1: all LN / transposes / K,V proj / lnTq =========
        with (
            tc.tile_pool(name="wqkv", bufs=1) as wkvpool,
            tc.tile_pool(name="cx", bufs=3) as cxpool,
            tc.tile_pool(name="lnout", bufs=6) as lnpool1,
            tc.tile_pool(name="lnTc", bufs=2) as lntcpool,
            tc.tile_pool(name="lntmpc", bufs=2) as lntmpc,
            tc.tile_pool(name="ps_ctx", bufs=3, space="PSUM") as ps_ctx,
            tc.tile_pool(name="ps_trc", bufs=3, space="PSUM") as ps_trc,
        ):
            wq_bf = wkvpool.tile([P, KS, DG], BF16, tag="wq_bf")
            wk_bf = wkvpool.tile([P, KS, DG], BF16, tag="wk_bf")
            wv_bf = wkvpool.tile([P, KS, DG], BF16, tag="wv_bf")
            nc.sync.dma_start(out=wq_bf,
                              in_=wq.rearrange("(s p) n -> p s n", p=P))
            nc.gpsimd.dma_start(out=wk_bf,
                                in_=wk.rearrange("(s p) n -> p s n", p=P))
            nc.sync.dma_start(out=wv_bf,
                              in_=wv.rearrange("(s p) n -> p s n", p=P))

            for c in range(NCH):
                xb = cxpool.tile([P, 4, D], F32, tag="xb")
                (nc.sync if c % 2 == 0 else nc.gpsimd).dma_start(
                    out=xb, in_=c_r[c])
                ln_tiles = ln_chunk(xb, gkv_b, btkv_b, lnpool1, lntmpc)
                lnT = lntcpool.tile([P, KS, 512], BF16, tag="lnT")
                transpose_chunk(ln_tiles, ps_trc, lnT)
                for m in range(NM):
                    pp = ps_ctx.tile([P, 512], F32, tag="pp")
                    for s in range(KS):
                        nc.tensor.matmul(pp, lhsT=wk_bf[:, s, m * P:(m + 1) * P],
                                         rhs=lnT[:, s, :],
                                         start=(s == 0), stop=(s == KS - 1))
                    if with_bias:
                        nc.scalar.activation(
                            out=kt[c][:, m, :], in_=pp,
                            func=mybir.ActivationFunctionType.Identity,
                            bias=bk_c[:, m:m + 1])
                    else:
                        nc.scalar.copy(out=kt[c][:, m, :], in_=pp)
                for tl in range(4):
                    t = 4 * c + tl
                    pp = ps_ctx.tile([P, 512], F32, tag="pp")
                    for s in range(KS):
                        nc.tensor.matmul(pp, lhsT=lnT[:, s, tl * P:(tl + 1) * P],
                                         rhs=wv_bf[:, s, :],
                                         start=(s == 0), stop=(s == KS - 1))
                    if with_bias:
                        nc.vector.tensor_tensor(
                            out=vs[:, t, :, 0:HD],
                            in0=pp.rearrange("p (h d) -> p h d", h=NH),
                            in1=bv_b.rearrange("p (h d) -> p h d", h=NH),
                            op=mybir.AluOpType.add)
                    else:
                        nc.vector.tensor_copy(
                            out=vs[:, t, :, 0:HD],
                            in_=pp.rearrange("p (h d) -> p h d", h=NH))

            # query chunks: LN + transpose + Q^T proj
            for c in range(NCH):
                xb = cxpool.tile([P, 4, D], F32, tag="xb",
                                 name=f"qxb{c}_{uq}")
                (nc.sync if c % 2 == 0 else nc.gpsimd).dma_start(
                    out=xb, in_=q_r[c])
                ln_tiles = ln_chunk(xb, gq_b, btq_b, lnpool1, lntmpc)
                lnT = lntcpool.tile([P, KS, 512], BF16, tag="lnT",
                                    name=f"qlnT{c}_{uq}")
                transpose_chunk(ln_tiles, ps_trc, lnT)
                for m in range(NM):
                    pp = ps_ctx.tile([P, 512], F32, tag="pp",
                                     name=f"qpp{c}_{m}_{uq}")
                    for s in range(KS):
                        nc.tensor.matmul(pp, lhsT=wq_bf[:, s, m * P:(m + 1) * P],
                                         rhs=lnT[:, s, :],
                                         start=(s == 0), stop=(s == KS - 1))
                    if with_bias:
                        nc.vector.tensor_scalar(
                            out=qt[c][:, m, :], in0=pp,
                            scalar1=bq_c[:, m:m + 1], scalar2=None,
                            op0=mybir.AluOpType.add)
                    else:
                        nc.vector.tensor_copy(out=qt[c][:, m, :], in_=pp)

        # ================= phase 2: pure attention =================
        with (
            tc.tile_pool(name="exp", bufs=exp_bufs) as exppool,
            tc.tile_pool(name="smalls", bufs=2) as smalls,
            tc.tile_pool(name="yout", bufs=2) as ypool,
            tc.tile_pool(name="ps_s", bufs=2, space="PSUM") as ps_s,
            tc.tile_pool(name="ps_av", bufs=2, space="PSUM") as ps_av,
            tc.tile_pool(name="ps_wo", bufs=2, space="PSUM") as ps_wo,
        ):

            def emit_av_chunk(prev, kg):
                c0, j0, exp_pair, avs = prev
                for hl in range(2):
                    for k2 in range(2):
                        ki = kg * 2 + k2
                        nc.tensor.matmul(avs[hl], lhsT=vs[:, ki, 2 * j0 + hl, :],
                                         rhs=exp_pair[hl][:, ki, :],
                                         start=(ki == 0), stop=(ki == NT - 1),
                                         skip_group_check=True)

            def emit_normalize(prev):
                c0, j0, exp_pair, avs = prev
                for hl in range(2):
                    av = avs[hl]
                    zsb = smalls.tile([1, 512], F32, tag="zsb",
                                      name=f"zsb{c0}_{j0}_{hl}_{uq}")
                    nc.vector.tensor_copy(out=zsb, in_=av[HD:HD + 1, :])
                    zrow = smalls.tile([1, 512], F32, tag="zrow",
                                       name=f"zrow{c0}_{j0}_{hl}_{uq}")
                    nc.vector.reciprocal_approx_fast(out=zrow, in_=zsb)
                    rinv = smalls.tile([HD, 512], F32, tag="rinv",
                                       name=f"rinv{c0}_{j0}_{hl}_{uq}")
                    nc.gpsimd.partition_broadcast(rinv, zrow)
                    nc.vector.tensor_tensor(
                        out=os_t[c0][hl * HD:(hl + 1) * HD, j0, :],
                        in0=av[0:HD, :], in1=rinv,
                        op=mybir.AluOpType.mult)

            def emit_wo_group(c0, g):
                tl, dc = g // 2, g % 2
                t = 4 * c0 + tl
                pp = ps_wo.tile([P, 512], F32, tag="pw",
                               name=f"wopp{c0}_{g}_{uq}")
                for m in range(NM):
                    nc.tensor.matmul(
                        pp, lhsT=os_t[c0][:, m, tl * P:(tl + 1) * P],
                        rhs=wo_bf[:, m, dc * 512:(dc + 1) * 512],
                        start=(m == 0), stop=(m == NM - 1),
                        skip_group_check=True)
                yt = ypool.tile([P, 512], F32, tag="y",
                                name=f"yt{c0}_{g}_{uq}")
                nc.vector.tensor_copy(out=yt, in_=pp)
                nc.sync.dma_start(
                    out=y_out[t * P:(t + 1) * P, dc * 512:(dc + 1) * 512],
                    in_=yt)

            if _LV == 0:
                return

            prev = None
            wo_pending = []   # (c, next_group_idx, appended_it)
            for c in range(NCH):
                for j in range(NM):
                    it = c * NM + j
                    exp_pair = [exppool.tile([P, NT, 512], BF16, tag=f"exp{hl}",
                                             name=f"exp{hl}_{c}_{j}_{uq}")
                                for hl in range(2)]
                    for kg in range(8):
                        ps_pair = [ps_s.tile([P, 2, 512], F32, tag="psS",
                                             name=f"psS{hl}_{c}_{j}_{kg}_{uq}")
                                   for hl in range(2)]
                        for k2 in range(2):
                            ki = kg * 2 + k2
                            for hl in range(2):
                                rows = slice(hl * HD, (hl + 1) * HD)
                                nc.tensor.matmul(
                                    ps_pair[hl][:, k2, :],
                                    lhsT=kt[ki // 4][rows, j,
                                              (ki % 4) * P:(ki % 4 + 1) * P],
                                    rhs=qt[c][rows, j, :],
                                    start=True, stop=True,
                                    skip_group_check=True)
                        for hl in range(2):
                            nc.scalar.activation(
                                out=exp_pair[hl][:, kg * 2:kg * 2 + 2, :],
                                in_=ps_pair[hl][:, :, :],
                                func=mybir.ActivationFunctionType.Exp,
                                scale=SCALE)
                        if prev is not None and _LV >= 2:
                            emit_av_chunk(prev, kg)
                        # Wo groups: only once the pending chunk's normalize has
                        # had >= a full iteration of PE runway (kg7 of it+1).
                        if _LV >= 4 and wo_pending and kg in (3, 7):
                            c0, g, ait = wo_pending[0]
                            if it > ait + 1 or (it == ait + 1 and kg == 7):
                                emit_wo_group(c0, g)
                                if g + 1 >= 8:
                                    wo_pending.pop(0)
                                else:
                                    wo_pending[0] = (c0, g + 1, ait)
                    if prev is not None and _LV >= 3:
                        emit_normalize(prev)
                        if prev[1] == NM - 1:      # finished batch-chunk prev[0]
                            wo_pending.append((prev[0], 0, it))
                    avs = [ps_av.tile([HD + 1, 512], F32, tag="av",
                                      name=f"av{c}_{j}_{hl}_{uq}")
                           for hl in range(2)]
                    prev = (c, j, exp_pair, avs)
            # drain: AV + normalize of the last (c,j), then remaining Wo groups
            if _LV >= 2:
                for kg in range(8):
                    emit_av_chunk(prev, kg)
            if _LV >= 3:
                emit_normalize(prev)
            if _LV >= 4:
                wo_pending.append((prev[0], 0, 0))
                for c0, g0, _ait in list(wo_pending):
                    for g in range(g0, 8):
                        emit_wo_group(c0, g)


_CACHE = {}


def _get_exec(ln_affine=True, with_bias=True, repeat=1, hw_loop=0,
              probe="full"):
    """Build the Bass program once and wrap it in a reusable jitted executor."""
    key = ("exec", ln_affine, with_bias, repeat, hw_loop, probe)
    if key in _CACHE:
        return _CACHE[key]

    import jax
    from jax.sharding import Mesh, PartitionSpec
    from jax.experimental.shard_map import shard_map
    from concourse import bass2jax

    nc = _build_program(ln_affine=ln_affine, with_bias=with_bias,
                        repeat=repeat, hw_loop=hw_loop, probe=probe)
    bass2jax.install_neuronx_cc_hook()

    partition_name = (nc.partition_id_tensor.name
                      if nc.partition_id_tensor else None)
    in_names, out_names, out_avals, zero_shapes = [], [], [], []
    in_dtypes = {}
    for alloc in nc.m.functions[0].allocations:
        if not isinstance(alloc, mybir.MemoryLocationSet):
            continue
        name = alloc.memorylocations[0].name
        if alloc.kind == "ExternalInput":
            if name != partition_name:
                in_names.append(name)
                in_dtypes[name] = mybir.dt.np(alloc.dtype)
        elif alloc.kind == "ExternalOutput":
            shape = tuple(alloc.tensor_shape)
            dtype = mybir.dt.np(alloc.dtype)
            out_names.append(name)
            out_avals.append(jax.core.ShapedArray(shape, dtype))
            zero_shapes.append((shape, dtype))
    n_params = len(in_names)
    n_outs = len(out_avals)
    all_names = list(in_names) + list(out_names)
    if partition_name is not None:
        all_names.append(partition_name)
    donate = tuple(range(n_params, n_params + n_outs))

    def _body(*args):
        operands = list(args)
        if partition_name is not None:
            operands.append(bass2jax.partition_id_tensor())
        outs = bass2jax._bass_exec_p.bind(
            *operands,
            out_avals=tuple(out_avals),
            in_names=tuple(all_names),
            out_names=tuple(out_names),
            lowering_input_output_aliases=(),
            sim_require_finite=True,
            sim_require_nnan=True,
            nc=nc,
        )
        return tuple(outs)

    n_cores = 8
    devices = jax.devices()[:n_cores]
    mesh = Mesh(np.asarray(devices), ("core",))
    in_specs = (PartitionSpec("core"),) * (n_params + n_outs)
    out_specs = (PartitionSpec("core"),) * n_outs
    sharded = jax.jit(
        shard_map(_body, mesh=mesh, in_specs=in_specs, out_specs=out_specs,
                  check_rep=False),
        donate_argnums=donate, keep_unused=True)

    def execute(in_maps):
        per_core = [[np.ascontiguousarray(np.asarray(m[name], in_dtypes[name]))
                     for name in in_names] for m in in_maps]
        concat_in = [np.concatenate([per_core[cc][i] for cc in range(n_cores)],
                                    axis=0) for i in range(n_params)]
        concat_zeros = [np.zeros((n_cores * s[0], *s[1:]), d)
                        for (s, d) in zero_shapes]
        out_arrs = sharded(*concat_in, *concat_zeros)
        return [
            {name: np.asarray(out_arrs[i]).reshape(n_cores, *out_avals[i].shape)[cc]
             for i, name in enumerate(out_names)}
            for cc in range(n_cores)
        ]

    _CACHE[key] = execute
    _CACHE[("parts", ln_affine, with_bias, repeat, hw_loop, probe)] = {
        "sharded": sharded, "in_names": in_names, "in_dtypes": in_dtypes,
        "n_params": n_params,
        "out_names": out_names, "out_avals": out_avals,
        "zero_shapes": zero_shapes, "mesh": mesh, "n_cores": n_cores,
        "body": _body, "in_specs": in_specs, "out_specs": out_specs,
        "donate": donate,
    }
    return execute


def _time_exec(in_maps, iters=5, ln_affine=True, with_bias=True,
               repeat=1, hw_loop=0, probe="full"):
    """Time the sharded executable with device-resident inputs (seconds)."""
    import time
    import jax
    from jax.sharding import NamedSharding, PartitionSpec

    _get_exec(ln_affine=ln_affine, with_bias=with_bias, repeat=repeat,
              hw_loop=hw_loop, probe=probe)
    parts = _CACHE[("parts", ln_affine, with_bias, repeat, hw_loop, probe)]
    sharded = parts["sharded"]
    n_cores = parts["n_cores"]
    in_dtypes = parts["in_dtypes"]
    sh = NamedSharding(parts["mesh"], PartitionSpec("core"))
    per_core = [[np.ascontiguousarray(np.asarray(m[name], in_dtypes[name]))
                 for name in parts["in_names"]] for m in in_maps]
    concat_in = [np.concatenate([per_core[cc][i] for cc in range(n_cores)],
                                axis=0) for i in range(parts["n_params"])]
    in_dev = [jax.device_put(a, sh) for a in concat_in]
    jax.block_until_ready(in_dev)
    times = []
    for _ in range(iters):
        z_dev = [jax.device_put(
                     np.zeros((n_cores * s[0], *s[1:]), d), sh)
                 for (s, d) in parts["zero_shapes"]]
        jax.block_until_ready(z_dev)
        t0 = time.perf_counter()
        out = sharded(*in_dev, *z_dev)
        jax.block_until_ready(out)
        times.append(time.perf_counter() - t0)
        del out
    return times


def _ln_is_identity(inputs):
    return all(
        np.all(np.asarray(inputs[k], np.float32) == v)
        for k, v in (("gq", 1.0), ("betq", 0.0), ("gkv", 1.0), ("betkv", 0.0))
    )


def _bias_is_zero(inputs):
    return all(
        np.all(np.asarray(inputs[k], np.float32) == 0.0)
        for k in ("bq", "bk", "bv")
    )


def _make_in_maps(inputs):
    import ml_dtypes
    BF = ml_dtypes.bfloat16
    q = np.asarray(inputs["query"], np.float32)
    c = np.asarray(inputs["context"], np.float32)
    Wq = np.asarray(inputs["Wq"], np.float32).astype(BF)
    Wk = np.asarray(inputs["Wk"], np.float32).astype(BF)
    Wv = np.asarray(inputs["Wv"], np.float32).astype(BF)
    Wo = np.asarray(inputs["Wo"], np.float32).astype(BF)
    bq = np.asarray(inputs["bq"], np.float32)
    bk = np.asarray(inputs["bk"], np.float32)
    bv = np.asarray(inputs["bv"], np.float32)
    gq = np.asarray(inputs["gq"], np.float32)
    btq = np.asarray(inputs["betq"], np.float32)
    gkv = np.asarray(inputs["gkv"], np.float32)
    btkv = np.asarray(inputs["betkv"], np.float32)
    ln_affine = not _ln_is_identity(inputs)
    with_bias = not _bias_is_zero(inputs)
    in_maps = []
    for core in range(8):
        b, hg = core // 2, core % 2
        sl = slice(hg * DG, (hg + 1) * DG)
        m = {
            "q_in": q[b], "c_in": c[b],
            "wq": Wq[:, sl], "wk": Wk[:, sl], "wv": Wv[:, sl],
            "wo": Wo[sl, :],
        }
        if with_bias:
            m.update({"bq": bq[sl], "bk": bk[sl], "bv": bv[sl]})
        if ln_affine:
            m.update({"gq": gq, "btq": btq, "gkv": gkv, "btkv": btkv})
        in_maps.append(m)
    return in_maps


def kernel(**inputs):
    ln_affine = not _ln_is_identity(inputs)
    with_bias = not _bias_is_zero(inputs)
    execute = _get_exec(ln_affine=ln_affine, with_bias=with_bias)
    in_maps = _make_in_maps(inputs)
    results = execute(in_maps)
    bo = np.asarray(inputs["bo"], np.float32)
    B = 4
    out = np.empty((B, N_TOK, D), np.float32)
    for b in range(B):
        out[b] = results[2 * b]["y_out"] + results[2 * b + 1]["y_out"] + bo
    return out


# revision 21
# speedup vs baseline: 1.0139x; 1.0139x over previous
"""Trainium2 Bass kernel for nn_CrossAttention (B=4, Nq=Nk=2048, D=1024, H=16).

Sharding: 8 cores = (batch b in 0..3) x (head-group hg in 0..1), 8 heads/core.
Each core gets its batch's query/context plus the column slice of Wq/Wk/Wv and
row slice of Wo for its 8 heads; LayerNorm params are replicated.  Host sums
the two head-group partial outputs per batch and adds bo.

Per-core pipeline (all matmuls bf16 with fp32 PSUM accumulation):
  Phase 1 (context): LN (fp32, bn_stats; rstd = exp(-0.5*ln(var+eps)) so the
  whole kernel uses ONE activation table set - no table-switch stalls)
  -> PE transpose -> K^T / V projections.  PSUM->SBUF moves ride ScalarE /
  DVE (ScalarE is otherwise idle here).
  Phase 2 (query proj + attention, software-pipelined): the q-chunk LN /
  transpose / Q^T projection for chunk c+1 is emitted inside the attention
  loop over chunk c, so DVE/PE/Pool chew projection work while ScalarE
  streams exp.  ScalarE does NOTHING but exp in this phase (the Q^T
  PSUM->SBUF moves ride the Pool engine).
  S^T = K Q^T tiles (2 heads packed in the 128-row PE array via auto
  tile_position row tiling -> concurrent matmul pairs on HW)
  -> exp on ScalarE with the 1/sqrt(dh) scale folded in
  -> AV matmul with M=65 (row 64 = softmax denominator Z, for free)
  -> normalize via reciprocal_approx_fast straight from PSUM + GPSIMD
  partition_broadcast + DVE -> Wo row-slice matmul (deferred half an
  iteration so the PE queue head never blocks on the normalize chain)
  -> fp32 partial output.
"""

import numpy as np

import concourse.bass as bass
import concourse.mybir as mybir
import concourse.tile as tile
from concourse import bacc
from concourse.masks import make_identity

P = 128
N_TOK = 2048          # tokens per batch (both Nq and Nk)
D = 1024              # model dim
KS = D // P           # 8 contraction subtiles
DG = 512              # per-core projection width (8 heads * 64)
NM = DG // P          # 4 output blocks / head-pair groups
NH = 8                # heads per core
HD = 64
NT = N_TOK // P       # 16 token tiles
NCH = N_TOK // 512    # 4 token chunks of 512
SCALE = HD ** -0.5
EPS = 1e-5

F32 = mybir.dt.float32
BF16 = mybir.dt.bfloat16
_UNIQ = [0]


def _build_program(ln_affine=True, with_bias=True, repeat=1, hw_loop=0,
                   probe="full"):
    nc = bacc.Bacc("TRN2", target_bir_lowering=False, debug=False)

    q_in = nc.dram_tensor("q_in", (N_TOK, D), F32, kind="ExternalInput")
    c_in = nc.dram_tensor("c_in", (N_TOK, D), F32, kind="ExternalInput")
    wq = nc.dram_tensor("wq", (D, DG), BF16, kind="ExternalInput")
    wk = nc.dram_tensor("wk", (D, DG), BF16, kind="ExternalInput")
    wv = nc.dram_tensor("wv", (D, DG), BF16, kind="ExternalInput")
    wo = nc.dram_tensor("wo", (DG, D), BF16, kind="ExternalInput")
    if with_bias:
        bq_d = nc.dram_tensor("bq", (DG,), F32, kind="ExternalInput")
        bk_d = nc.dram_tensor("bk", (DG,), F32, kind="ExternalInput")
        bv_d = nc.dram_tensor("bv", (DG,), F32, kind="ExternalInput")
    else:
        bq_d = bk_d = bv_d = None
    if ln_affine:
        gq_d = nc.dram_tensor("gq", (D,), F32, kind="ExternalInput")
        btq_d = nc.dram_tensor("btq", (D,), F32, kind="ExternalInput")
        gkv_d = nc.dram_tensor("gkv", (D,), F32, kind="ExternalInput")
        btkv_d = nc.dram_tensor("btkv", (D,), F32, kind="ExternalInput")
    else:
        gq_d = btq_d = gkv_d = btkv_d = None
    y_out = nc.dram_tensor("y_out", (N_TOK, D), F32, kind="ExternalOutput")

    import contextlib

    with tile.TileContext(nc) as tc:
        loop_ctx = tc.For_i(0, hw_loop, 1) if hw_loop else None
        with (loop_ctx if loop_ctx is not None else contextlib.nullcontext()):
         for _rep in range(repeat):
            _UNIQ[0] += 1
            _emit_kernel(nc, tc, q_in, c_in, wq, wk, wv, wo,
                         bq_d, bk_d, bv_d, gq_d, btq_d, gkv_d, btkv_d,
                         y_out, ln_affine, with_bias, probe)

    nc.finalize()
    return nc


def _emit_kernel(nc, tc, q_in, c_in, wq, wk, wv, wo,
                 bq_d, bk_d, bv_d, gq_d, btq_d, gkv_d, btkv_d,
                 y_out, ln_affine, with_bias, probe="full"):
    # probe: "ctx" (phase 1 + qproj only), "sexp" (+ S/exp), "av" (+ AV),
    #        "norm" (+ normalize), "full"
    _LV = {"ctx": 0, "sexp": 1, "av": 2, "norm": 3, "full": 4}[probe]
    uq = _UNIQ[0]
    exp_bufs = 1 if ln_affine else 3
    with (
        tc.tile_pool(name="persist", bufs=1) as persist,
        tc.tile_pool(name="wqo", bufs=1) as wqo,
        tc.tile_pool(name="consts", bufs=1) as consts,
        tc.tile_pool(name="stats", bufs=4) as stats,
    ):
        # ---------------- persistent tensors ----------------
        qt = [persist.tile([P, NM, 512], BF16, tag=f"qt{c}", name=f"qt{c}_{uq}")
              for c in range(NCH)]   # Q^T per token chunk
        kt = [persist.tile([P, NM, 512], BF16, tag=f"kt{c}", name=f"kt{c}_{uq}")
              for c in range(NCH)]   # K^T per key chunk
        vs = persist.tile([P, NT, NH, HD + 1], BF16, tag="vs")
        os_t = [persist.tile([P, NM, 512], BF16, tag=f"os{c}",
                             name=f"os{c}_{uq}")
                for c in range(NCH)]
        nc.vector.memset(vs[:, :, :, HD:HD + 1], 1.0)
        wo_bf = wqo.tile([P, NM, D], BF16, tag="wo_bf")

        # ---------------- constants ----------------
        ident = consts.tile([P, P], BF16, tag="ident")
        make_identity(nc, ident)
        eps_t = consts.tile([P, 1], F32, tag="eps")
        nc.vector.memset(eps_t, EPS)
        # DVE bit-trick exp constants: bf16bits(exp(s*SCALE)) ~ int16(A*s + B)
        ab_t = consts.tile([P, 2], F32, tag="ab")
        nc.vector.memset(ab_t[:, 0:1], 128.0 * 1.4426950408889634 * SCALE)
        nc.vector.memset(ab_t[:, 1:2], 127.0 * 128.0 - 7.0)
        if ln_affine:
            gq_b = consts.tile([P, D], F32, tag="gq_b")
            nc.gpsimd.dma_start(out=gq_b,
                                in_=gq_d[None, :].to_broadcast((P, D)))
            btq_b = consts.tile([P, D], F32, tag="btq_b")
            nc.gpsimd.dma_start(out=btq_b,
                                in_=btq_d[None, :].to_broadcast((P, D)))
            gkv_b = consts.tile([P, D], F32, tag="gkv_b")
            nc.gpsimd.dma_start(out=gkv_b,
                                in_=gkv_d[None, :].to_broadcast((P, D)))
            btkv_b = consts.tile([P, D], F32, tag="btkv_b")
            nc.gpsimd.dma_start(out=btkv_b,
                                in_=btkv_d[None, :].to_broadcast((P, D)))
        else:
            gq_b = btq_b = gkv_b = btkv_b = None
        if with_bias:
            bv_b = consts.tile([P, DG], F32, tag="bv_b")
            nc.gpsimd.dma_start(out=bv_b,
                                in_=bv_d[None, :].to_broadcast((P, DG)))
            bq_c = consts.tile([P, NM], F32, tag="bq_c")
            nc.sync.dma_start(out=bq_c,
                              in_=bq_d.rearrange("(m p) -> p m", p=P))
            bk_c = consts.tile([P, NM], F32, tag="bk_c")
            nc.sync.dma_start(out=bk_c,
                              in_=bk_d.rearrange("(m p) -> p m", p=P))
        else:
            bv_b = bq_c = bk_c = None

        nc.gpsimd.dma_start(out=wo_bf,
                            in_=wo.rearrange("(m p) n -> p m n", p=P))

        c_r = c_in.rearrange("(n i p) d -> n p i d", p=P, i=4)
        q_r = q_in.rearrange("(n i p) d -> n p i d", p=P, i=4)

        lncnt = [0]

        def ln_chunk(xb, g_b, b_b, lnpool, lntmp=None):
            """LN 4 token tiles xb[:, tl, :] -> list of [128, 1024] bf16."""
            lncnt[0] += 1
            mv = stats.tile([P, 4, 2], F32, tag="mv")
            for tl in range(4):
                st = stats.tile([P, 2, 6], F32, tag="bnst")
                nc.vector.bn_stats(out=st[:, 0, :], in_=xb[:, tl, 0:512])
                nc.vector.bn_stats(out=st[:, 1, :], in_=xb[:, tl, 512:1024])
                nc.vector.bn_aggr(out=mv[:, tl, :], in_=st)
            # rstd = exp(-0.5 * ln(var + eps)) -- stays in the exp table set
            lnv = stats.tile([P, 4], F32, tag="lnv")
            nc.scalar.activation(out=lnv, in_=mv[:, :, 1],
                                 func=mybir.ActivationFunctionType.Ln,
                                 bias=eps_t)
            rstd = stats.tile([P, 4], F32, tag="rstd")
            nc.scalar.activation(out=rstd, in_=lnv,
                                 func=mybir.ActivationFunctionType.Exp,
                                 scale=-0.5)
            lnts = []
            for tl in range(4):
                x = xb[:, tl, :]
                lnt = lnpool.tile([P, D], BF16, tag="ln",
                                  name=f"ln{tl}_{uq}_{lncnt[0]}")
                if not ln_affine:
                    nc.vector.tensor_scalar(
                        out=lnt, in0=x, scalar1=mv[:, tl, 0:1],
                        scalar2=rstd[:, tl:tl + 1],
                        op0=mybir.AluOpType.subtract,
                        op1=mybir.AluOpType.mult)
                else:
                    xc = lntmp.tile([P, D], F32, tag="xc")
                    nc.vector.tensor_scalar(
                        out=xc, in0=x, scalar1=mv[:, tl, 0:1],
                        scalar2=rstd[:, tl:tl + 1],
                        op0=mybir.AluOpType.subtract,
                        op1=mybir.AluOpType.mult)
                    xg = lntmp.tile([P, D], F32, tag="xg")
                    nc.vector.tensor_tensor(out=xg, in0=xc, in1=g_b,
                                            op=mybir.AluOpType.mult)
                    nc.vector.tensor_tensor(out=lnt, in0=xg, in1=b_b,
                                            op=mybir.AluOpType.add)
                lnts.append(lnt)
            return lnts

        def transpose_chunk(ln_tiles, ps_pool, lnT):
            """4 LN tiles ([128 tok, 1024 feat]) -> lnT [128 feat, 8, 512 tok]."""
            for s in range(KS):
                pt = ps_pool.tile([P, 512], BF16, tag="tr")
                for tl in range(4):
                    nc.tensor.transpose(pt[:, tl * P:(tl + 1) * P],
                                        ln_tiles[tl][:, s * P:(s + 1) * P],
                                        ident)
                nc.scalar.copy(out=lnT[:, s, :], in_=pt)
            return lnT

        # ========= phase 1: all LN / transposes / K,V proj / lnTq =========
        with (
            tc.tile_pool(name="wqkv", bufs=1) as wkvpool,
            tc.tile_pool(name="cx", bufs=3) as cxpool,
            tc.tile_pool(name="lnout", bufs=6) as lnpool1,
            tc.tile_pool(name="lnTc", bufs=2) as lntcpool,
            tc.tile_pool(name="lntmpc", bufs=2) as lntmpc,
            tc.tile_pool(name="ps_ctx", bufs=3, space="PSUM") as ps_ctx,
            tc.tile_pool(name="ps_trc", bufs=3, space="PSUM") as ps_trc,
        ):
            wq_bf = wkvpool.tile([P, KS, DG], BF16, tag="wq_bf")
            wk_bf = wkvpool.tile([P, KS, DG], BF16, tag="wk_bf")
            wv_bf = wkvpool.tile([P, KS, DG], BF16, tag="wv_bf")
            nc.sync.dma_start(out=wq_bf,
                              in_=wq.rearrange("(s p) n -> p s n", p=P))
            nc.gpsimd.dma_start(out=wk_bf,
                                in_=wk.rearrange("(s p) n -> p s n", p=P))
            nc.sync.dma_start(out=wv_bf,
                              in_=wv.rearrange("(s p) n -> p s n", p=P))

            for c in range(NCH):
                xb = cxpool.tile([P, 4, D], F32, tag="xb")
                (nc.sync if c % 2 == 0 else nc.gpsimd).dma_start(
                    out=xb, in_=c_r[c])
                ln_tiles = ln_chunk(xb, gkv_b, btkv_b, lnpool1, lntmpc)
                lnT = lntcpool.tile([P, KS, 512], BF16, tag="lnT")
                transpose_chunk(ln_tiles, ps_trc, lnT)
                for m in range(NM):
                    pp = ps_ctx.tile([P, 512], F32, tag="pp")
                    for s in range(KS):
                        nc.tensor.matmul(pp, lhsT=wk_bf[:, s, m * P:(m + 1) * P],
                                         rhs=lnT[:, s, :],
                                         start=(s == 0), stop=(s == KS - 1))
                    if with_bias:
                        nc.scalar.activation(
                            out=kt[c][:, m, :], in_=pp,
                            func=mybir.ActivationFunctionType.Identity,
                            bias=bk_c[:, m:m + 1])
                    else:
                        nc.scalar.copy(out=kt[c][:, m, :], in_=pp)
                for tl in range(4):
                    t = 4 * c + tl
                    pp = ps_ctx.tile([P, 512], F32, tag="pp")
                    for s in range(KS):
                        nc.tensor.matmul(pp, lhsT=lnT[:, s, tl * P:(tl + 1) * P],
                                         rhs=wv_bf[:, s, :],
                                         start=(s == 0), stop=(s == KS - 1))
                    if with_bias:
                        nc.vector.tensor_tensor(
                            out=vs[:, t, :, 0:HD],
                            in0=pp.rearrange("p (h d) -> p h d", h=NH),
                            in1=bv_b.rearrange("p (h d) -> p h d", h=NH),
                            op=mybir.AluOpType.add)
                    else:
                        nc.vector.tensor_copy(
                            out=vs[:, t, :, 0:HD],
                            in_=pp.rearrange("p (h d) -> p h d", h=NH))

            # query chunks: LN + transpose + Q^T proj
            for c in range(NCH):
                xb = cxpool.tile([P, 4, D], F32, tag="xb",
                                 name=f"qxb{c}_{uq}")
                (nc.sync if c % 2 == 0 else nc.gpsimd).dma_start(
                    out=xb, in_=q_r[c])
                ln_tiles = ln_chunk(xb, gq_b, btq_b, lnpool1, lntmpc)
                lnT = lntcpool.tile([P, KS, 512], BF16, tag="lnT",
                                    name=f"qlnT{c}_{uq}")
                transpose_chunk(ln_tiles, ps_trc, lnT)
                for m in range(NM):
                    pp = ps_ctx.tile([P, 512], F32, tag="pp",
                                     name=f"qpp{c}_{m}_{uq}")
                    for s in range(KS):
                        nc.tensor.matmul(pp, lhsT=wq_bf[:, s, m * P:(m + 1) * P],
                                         rhs=lnT[:, s, :],
                                         start=(s == 0), stop=(s == KS - 1))
                    if with_bias:
                        nc.vector.tensor_scalar(
                            out=qt[c][:, m, :], in0=pp,
                            scalar1=bq_c[:, m:m + 1], scalar2=None,
                            op0=mybir.AluOpType.add)
                    else:
                        nc.vector.tensor_copy(out=qt[c][:, m, :], in_=pp)

        # ================= phase 2: pure attention =================
        with (
            tc.tile_pool(name="exp", bufs=exp_bufs) as exppool,
            tc.tile_pool(name="smalls", bufs=2) as smalls,
            tc.tile_pool(name="yout", bufs=2) as ypool,
            tc.tile_pool(name="ps_s", bufs=2, space="PSUM") as ps_s,
            tc.tile_pool(name="ps_av", bufs=2, space="PSUM") as ps_av,
            tc.tile_pool(name="ps_wo", bufs=2, space="PSUM") as ps_wo,
        ):

            def emit_av_chunk(prev, kg):
                c0, j0, exp_pair, avs = prev
                for hl in range(2):
                    for k2 in range(2):
                        ki = kg * 2 + k2
                        nc.tensor.matmul(avs[hl], lhsT=vs[:, ki, 2 * j0 + hl, :],
                                         rhs=exp_pair[hl][:, ki, :],
                                         start=(ki == 0), stop=(ki == NT - 1),
                                         skip_group_check=True)

            def emit_normalize(prev):
                c0, j0, exp_pair, avs = prev
                for hl in range(2):
                    av = avs[hl]
                    zsb = smalls.tile([1, 512], F32, tag="zsb",
                                      name=f"zsb{c0}_{j0}_{hl}_{uq}")
                    nc.vector.tensor_copy(out=zsb, in_=av[HD:HD + 1, :])
                    zrow = smalls.tile([1, 512], F32, tag="zrow",
                                       name=f"zrow{c0}_{j0}_{hl}_{uq}")
                    nc.vector.reciprocal_approx_fast(out=zrow, in_=zsb)
                    rinv = smalls.tile([HD, 512], F32, tag="rinv",
                                       name=f"rinv{c0}_{j0}_{hl}_{uq}")
                    nc.gpsimd.partition_broadcast(rinv, zrow)
                    nc.vector.tensor_tensor(
                        out=os_t[c0][hl * HD:(hl + 1) * HD, j0, :],
                        in0=av[0:HD, :], in1=rinv,
                        op=mybir.AluOpType.mult)

            def emit_wo_group(c0, g):
                tl, dc = g // 2, g % 2
                t = 4 * c0 + tl
                pp = ps_wo.tile([P, 512], F32, tag="pw",
                               name=f"wopp{c0}_{g}_{uq}")
                for m in range(NM):
                    nc.tensor.matmul(
                        pp, lhsT=os_t[c0][:, m, tl * P:(tl + 1) * P],
                        rhs=wo_bf[:, m, dc * 512:(dc + 1) * 512],
                        start=(m == 0), stop=(m == NM - 1),
                        skip_group_check=True)
                yt = ypool.tile([P, 512], F32, tag="y",
                                name=f"yt{c0}_{g}_{uq}")
                nc.vector.tensor_copy(out=yt, in_=pp)
                nc.sync.dma_start(
                    out=y_out[t * P:(t + 1) * P, dc * 512:(dc + 1) * 512],
                    in_=yt)

            if _LV == 0:
                return

            prev = None
            wo_pending = []   # (c, next_group_idx, appended_it)
            for c in range(NCH):
                for j in range(NM):
                    it = c * NM + j
                    exp_pair = [exppool.tile([P, NT, 512], BF16, tag=f"exp{hl}",
                                             name=f"exp{hl}_{c}_{j}_{uq}")
                                for hl in range(2)]
                    for kg in range(8):
                        ps_pair = [ps_s.tile([P, 2, 512], F32, tag="psS",
                                             name=f"psS{hl}_{c}_{j}_{kg}_{uq}")
                                   for hl in range(2)]
                        for k2 in range(2):
                            ki = kg * 2 + k2
                            for hl in range(2):
                                rows = slice(hl * HD, (hl + 1) * HD)
                                nc.tensor.matmul(
                                    ps_pair[hl][:, k2, :],
                                    lhsT=kt[ki // 4][rows, j,
                                              (ki % 4) * P:(ki % 4 + 1) * P],
                                    rhs=qt[c][rows, j, :],
                                    start=True, stop=True,
                                    skip_group_check=True)
                        for hl in range(2):
                            if kg in (2, 5):
                                # Schraudolph-style exp on DVE: write the bf16
                                # bit pattern as an int16 value convert
                                nc.vector.tensor_scalar(
                                    out=exp_pair[hl][:, kg * 2:kg * 2 + 2, :]
                                        .bitcast(mybir.dt.int16),
                                    in0=ps_pair[hl][:, :, :],
                                    scalar1=ab_t[:, 0:1],
                                    scalar2=ab_t[:, 1:2],
                                    op0=mybir.AluOpType.mult,
                                    op1=mybir.AluOpType.add)
                            else:
                                nc.scalar.activation(
                                    out=exp_pair[hl][:, kg * 2:kg * 2 + 2, :],
                                    in_=ps_pair[hl][:, :, :],
                                    func=mybir.ActivationFunctionType.Exp,
                                    scale=SCALE)
                        if prev is not None and _LV >= 2:
                            emit_av_chunk(prev, kg)
                        # Wo groups: only once the pending chunk's normalize has
                        # had >= a full iteration of PE runway (kg7 of it+1).
                        if _LV >= 4 and wo_pending and kg in (3, 7):
                            c0, g, ait = wo_pending[0]
                            if it > ait + 1 or (it == ait + 1 and kg == 7):
                                emit_wo_group(c0, g)
                                if g + 1 >= 8:
                                    wo_pending.pop(0)
                                else:
                                    wo_pending[0] = (c0, g + 1, ait)
                    if prev is not None and _LV >= 3:
                        emit_normalize(prev)
                        if prev[1] == NM - 1:      # finished batch-chunk prev[0]
                            wo_pending.append((prev[0], 0, it))
                    avs = [ps_av.tile([HD + 1, 512], F32, tag="av",
                                      name=f"av{c}_{j}_{hl}_{uq}")
                           for hl in range(2)]
                    prev = (c, j, exp_pair, avs)
            # drain: AV + normalize of the last (c,j), then remaining Wo groups
            if _LV >= 2:
                for kg in range(8):
                    emit_av_chunk(prev, kg)
            if _LV >= 3:
                emit_normalize(prev)
            if _LV >= 4:
                wo_pending.append((prev[0], 0, 0))
                for c0, g0, _ait in list(wo_pending):
                    for g in range(g0, 8):
                        emit_wo_group(c0, g)


_CACHE = {}


def _get_exec(ln_affine=True, with_bias=True, repeat=1, hw_loop=0,
              probe="full"):
    """Build the Bass program once and wrap it in a reusable jitted executor."""
    key = ("exec", ln_affine, with_bias, repeat, hw_loop, probe)
    if key in _CACHE:
        return _CACHE[key]

    import jax
    from jax.sharding import Mesh, PartitionSpec
    from jax.experimental.shard_map import shard_map
    from concourse import bass2jax

    nc = _build_program(ln_affine=ln_affine, with_bias=with_bias,
                        repeat=repeat, hw_loop=hw_loop, probe=probe)
    bass2jax.install_neuronx_cc_hook()

    partition_name = (nc.partition_id_tensor.name
                      if nc.partition_id_tensor else None)
    in_names, out_names, out_avals, zero_shapes = [], [], [], []
    in_dtypes = {}
    for alloc in nc.m.functions[0].allocations:
        if not isinstance(alloc, mybir.MemoryLocationSet):
            continue
        name = alloc.memorylocations[0].name
        if alloc.kind == "ExternalInput":
            if name != partition_name:
                in_names.append(name)
                in_dtypes[name] = mybir.dt.np(alloc.dtype)
        elif alloc.kind == "ExternalOutput":
            shape = tuple(alloc.tensor_shape)
            dtype = mybir.dt.np(alloc.dtype)
            out_names.append(name)
            out_avals.append(jax.core.ShapedArray(shape, dtype))
            zero_shapes.append((shape, dtype))
    n_params = len(in_names)
    n_outs = len(out_avals)
    all_names = list(in_names) + list(out_names)
    if partition_name is not None:
        all_names.append(partition_name)
    donate = tuple(range(n_params, n_params + n_outs))

    def _body(*args):
        operands = list(args)
        if partition_name is not None:
            operands.append(bass2jax.partition_id_tensor())
        outs = bass2jax._bass_exec_p.bind(
            *operands,
            out_avals=tuple(out_avals),
            in_names=tuple(all_names),
            out_names=tuple(out_names),
            lowering_input_output_aliases=(),
            sim_require_finite=True,
            sim_require_nnan=True,
            nc=nc,
        )
        return tuple(outs)

    n_cores = 8
    devices = jax.devices()[:n_cores]
    mesh = Mesh(np.asarray(devices), ("core",))
    in_specs = (PartitionSpec("core"),) * (n_params + n_outs)
    out_specs = (PartitionSpec("core"),) * n_outs
    sharded = jax.jit(
        shard_map(_body, mesh=mesh, in_specs=in_specs, out_specs=out_specs,
                  check_rep=False),
        donate_argnums=donate, keep_unused=True)

    def execute(in_maps):
        per_core = [[np.ascontiguousarray(np.asarray(m[name], in_dtypes[name]))
                     for name in in_names] for m in in_maps]
        concat_in = [np.concatenate([per_core[cc][i] for cc in range(n_cores)],
                                    axis=0) for i in range(n_params)]
        concat_zeros = [np.zeros((n_cores * s[0], *s[1:]), d)
                        for (s, d) in zero_shapes]
        out_arrs = sharded(*concat_in, *concat_zeros)
        return [
            {name: np.asarray(out_arrs[i]).reshape(n_cores, *out_avals[i].shape)[cc]
             for i, name in enumerate(out_names)}
            for cc in range(n_cores)
        ]

    _CACHE[key] = execute
    _CACHE[("parts", ln_affine, with_bias, repeat, hw_loop, probe)] = {
        "sharded": sharded, "in_names": in_names, "in_dtypes": in_dtypes,
        "n_params": n_params,
        "out_names": out_names, "out_avals": out_avals,
        "zero_shapes": zero_shapes, "mesh": mesh, "n_cores": n_cores,
        "body": _body, "in_specs": in_specs, "out_specs": out_specs,
        "donate": donate,
    }
    return execute


def _time_exec(in_maps, iters=5, ln_affine=True, with_bias=True,
               repeat=1, hw_loop=0, probe="full"):
    """Time the sharded executable with device-resident inputs (seconds)."""
    import time
    import jax
    from jax.sharding import NamedSharding, PartitionSpec

    _get_exec(ln_affine=ln_affine, with_bias=with_bias, repeat=repeat,
              hw_loop=hw_loop, probe=probe)
    parts = _CACHE[("parts", ln_affine, with_bias, repeat, hw_loop, probe)]
    sharded = parts["sharded"]
    n_cores = parts["n_cores"]
    in_dtypes = parts["in_dtypes"]
    sh = NamedSharding(parts["mesh"], PartitionSpec("core"))
    per_core = [[np.ascontiguousarray(np.asarray(m[name], in_dtypes[name]))
                 for name in parts["in_names"]] for m in in_maps]
    concat_in = [np.concatenate([per_core[cc][i] for cc in range(n_cores)],
                                axis=0) for i in range(parts["n_params"])]
    in_dev = [jax.device_put(a, sh) for a in concat_in]
    jax.block_until_ready(in_dev)
    times = []
    for _ in range(iters):
        z_dev = [jax.device_put(
                     np.zeros((n_cores * s[0], *s[1:]), d), sh)
                 for (s, d) in parts["zero_shapes"]]
        jax.block_until_ready(z_dev)
        t0 = time.perf_counter()
        out = sharded(*in_dev, *z_dev)
        jax.block_until_ready(out)
        times.append(time.perf_counter() - t0)
        del out
    return times


def _ln_is_identity(inputs):
    return all(
        np.all(np.asarray(inputs[k], np.float32) == v)
        for k, v in (("gq", 1.0), ("betq", 0.0), ("gkv", 1.0), ("betkv", 0.0))
    )


def _bias_is_zero(inputs):
    return all(
        np.all(np.asarray(inputs[k], np.float32) == 0.0)
        for k in ("bq", "bk", "bv")
    )


def _make_in_maps(inputs):
    import ml_dtypes
    BF = ml_dtypes.bfloat16
    q = np.asarray(inputs["query"], np.float32)
    c = np.asarray(inputs["context"], np.float32)
    Wq = np.asarray(inputs["Wq"], np.float32).astype(BF)
    Wk = np.asarray(inputs["Wk"], np.float32).astype(BF)
    Wv = np.asarray(inputs["Wv"], np.float32).astype(BF)
    Wo = np.asarray(inputs["Wo"], np.float32).astype(BF)
    bq = np.asarray(inputs["bq"], np.float32)
    bk = np.asarray(inputs["bk"], np.float32)
    bv = np.asarray(inputs["bv"], np.float32)
    gq = np.asarray(inputs["gq"], np.float32)
    btq = np.asarray(inputs["betq"], np.float32)
    gkv = np.asarray(inputs["gkv"], np.float32)
    btkv = np.asarray(inputs["betkv"], np.float32)
    ln_affine = not _ln_is_identity(inputs)
    with_bias = not _bias_is_zero(inputs)
    in_maps = []
    for core in range(8):
        b, hg = core // 2, core % 2
        sl = slice(hg * DG, (hg + 1) * DG)
        m = {
            "q_in": q[b], "c_in": c[b],
            "wq": Wq[:, sl], "wk": Wk[:, sl], "wv": Wv[:, sl],
            "wo": Wo[sl, :],
        }
        if with_bias:
            m.update({"bq": bq[sl], "bk": bk[sl], "bv": bv[sl]})
        if ln_affine:
            m.update({"gq": gq, "btq": btq, "gkv": gkv, "btkv": btkv})
        in_maps.append(m)
    return in_maps


def kernel(**inputs):
    ln_affine = not _ln_is_identity(inputs)
    with_bias = not _bias_is_zero(inputs)
    execute = _get_exec(ln_affine=ln_affine, with_bias=with_bias)
    in_maps = _make_in_maps(inputs)
    results = execute(in_maps)
    bo = np.asarray(inputs["bo"], np.float32)
    B = 4
    out = np.empty((B, N_TOK, D), np.float32)
    for b in range(B):
        out[b] = results[2 * b]["y_out"] + results[2 * b + 1]["y_out"] + bo
    return out


# revision 23
# speedup vs baseline: 1.0319x; 1.0177x over previous
"""Trainium2 Bass kernel for nn_CrossAttention (B=4, Nq=Nk=2048, D=1024, H=16).

Sharding: 8 cores = (batch b in 0..3) x (head-group hg in 0..1), 8 heads/core.
Each core gets its batch's query/context plus the column slice of Wq/Wk/Wv and
row slice of Wo for its 8 heads; LayerNorm params are replicated.  Host sums
the two head-group partial outputs per batch and adds bo.

Per-core pipeline (all matmuls bf16 with fp32 PSUM accumulation):
  Phase 1 (context): LN (fp32, bn_stats; rstd = exp(-0.5*ln(var+eps)) so the
  whole kernel uses ONE activation table set - no table-switch stalls)
  -> PE transpose -> K^T / V projections.  PSUM->SBUF moves ride ScalarE /
  DVE (ScalarE is otherwise idle here).
  Phase 2 (query proj + attention, software-pipelined): the q-chunk LN /
  transpose / Q^T projection for chunk c+1 is emitted inside the attention
  loop over chunk c, so DVE/PE/Pool chew projection work while ScalarE
  streams exp.  ScalarE does NOTHING but exp in this phase (the Q^T
  PSUM->SBUF moves ride the Pool engine).
  S^T = K Q^T tiles (2 heads packed in the 128-row PE array via auto
  tile_position row tiling -> concurrent matmul pairs on HW)
  -> exp on ScalarE with the 1/sqrt(dh) scale folded in
  -> AV matmul with M=65 (row 64 = softmax denominator Z, for free)
  -> normalize via reciprocal_approx_fast straight from PSUM + GPSIMD
  partition_broadcast + DVE -> Wo row-slice matmul (deferred half an
  iteration so the PE queue head never blocks on the normalize chain)
  -> fp32 partial output.
"""

import numpy as np

import concourse.bass as bass
import concourse.mybir as mybir
import concourse.tile as tile
from concourse import bacc
from concourse.masks import make_identity

P = 128
N_TOK = 2048          # tokens per batch (both Nq and Nk)
D = 1024              # model dim
KS = D // P           # 8 contraction subtiles
DG = 512              # per-core projection width (8 heads * 64)
NM = DG // P          # 4 output blocks / head-pair groups
NH = 8                # heads per core
HD = 64
NT = N_TOK // P       # 16 token tiles
NCH = N_TOK // 512    # 4 token chunks of 512
SCALE = HD ** -0.5
EPS = 1e-5

F32 = mybir.dt.float32
BF16 = mybir.dt.bfloat16
_UNIQ = [0]


def _build_program(ln_affine=True, with_bias=True, repeat=1, hw_loop=0,
                   probe="full"):
    nc = bacc.Bacc("TRN2", target_bir_lowering=False, debug=False)

    q_in = nc.dram_tensor("q_in", (N_TOK, D), F32, kind="ExternalInput")
    c_in = nc.dram_tensor("c_in", (N_TOK, D), F32, kind="ExternalInput")
    wq = nc.dram_tensor("wq", (D, DG), BF16, kind="ExternalInput")
    wk = nc.dram_tensor("wk", (D, DG), BF16, kind="ExternalInput")
    wv = nc.dram_tensor("wv", (D, DG), BF16, kind="ExternalInput")
    wo = nc.dram_tensor("wo", (DG, D), BF16, kind="ExternalInput")
    if with_bias:
        bq_d = nc.dram_tensor("bq", (DG,), F32, kind="ExternalInput")
        bk_d = nc.dram_tensor("bk", (DG,), F32, kind="ExternalInput")
        bv_d = nc.dram_tensor("bv", (DG,), F32, kind="ExternalInput")
    else:
        bq_d = bk_d = bv_d = None
    if ln_affine:
        gq_d = nc.dram_tensor("gq", (D,), F32, kind="ExternalInput")
        btq_d = nc.dram_tensor("btq", (D,), F32, kind="ExternalInput")
        gkv_d = nc.dram_tensor("gkv", (D,), F32, kind="ExternalInput")
        btkv_d = nc.dram_tensor("btkv", (D,), F32, kind="ExternalInput")
    else:
        gq_d = btq_d = gkv_d = btkv_d = None
    y_out = nc.dram_tensor("y_out", (N_TOK, D), F32, kind="ExternalOutput")

    import contextlib

    with tile.TileContext(nc) as tc:
        loop_ctx = tc.For_i(0, hw_loop, 1) if hw_loop else None
        with (loop_ctx if loop_ctx is not None else contextlib.nullcontext()):
         for _rep in range(repeat):
            _UNIQ[0] += 1
            _emit_kernel(nc, tc, q_in, c_in, wq, wk, wv, wo,
                         bq_d, bk_d, bv_d, gq_d, btq_d, gkv_d, btkv_d,
                         y_out, ln_affine, with_bias, probe)

    nc.finalize()
    return nc


def _emit_kernel(nc, tc, q_in, c_in, wq, wk, wv, wo,
                 bq_d, bk_d, bv_d, gq_d, btq_d, gkv_d, btkv_d,
                 y_out, ln_affine, with_bias, probe="full"):
    # probe: "ctx" (phase 1 + qproj only), "sexp" (+ S/exp), "av" (+ AV),
    #        "norm" (+ normalize), "full"
    _LV = {"ctx": 0, "sexp": 1, "av": 2, "norm": 3, "full": 4}[probe]
    uq = _UNIQ[0]
    exp_bufs = 1 if ln_affine else 3
    with (
        tc.tile_pool(name="persist", bufs=1) as persist,
        tc.tile_pool(name="wqo", bufs=1) as wqo,
        tc.tile_pool(name="consts", bufs=1) as consts,
        tc.tile_pool(name="stats", bufs=4) as stats,
    ):
        # ---------------- persistent tensors ----------------
        qt = [persist.tile([P, NM, 512], BF16, tag=f"qt{c}", name=f"qt{c}_{uq}")
              for c in range(NCH)]   # Q^T per token chunk
        kt = [persist.tile([P, NM, 512], BF16, tag=f"kt{c}", name=f"kt{c}_{uq}")
              for c in range(NCH)]   # K^T per key chunk
        vs = persist.tile([P, NT, NH, HD + 1], BF16, tag="vs")
        os_t = [persist.tile([P, NM, 512], BF16, tag=f"os{c}",
                             name=f"os{c}_{uq}")
                for c in range(NCH)]
        nc.vector.memset(vs[:, :, :, HD:HD + 1], 1.0)
        wo_bf = wqo.tile([P, NM, D], BF16, tag="wo_bf")

        # ---------------- constants ----------------
        ident = consts.tile([P, P], BF16, tag="ident")
        make_identity(nc, ident)
        eps_t = consts.tile([P, 1], F32, tag="eps")
        nc.vector.memset(eps_t, EPS)
        # DVE bit-trick exp constants: bf16bits(exp(s*SCALE)) ~ int16(A*s + B)
        ab_t = consts.tile([P, 2], F32, tag="ab")
        nc.vector.memset(ab_t[:, 0:1], 128.0 * 1.4426950408889634 * SCALE)
        nc.vector.memset(ab_t[:, 1:2], 127.0 * 128.0 - 7.0)
        if ln_affine:
            gq_b = consts.tile([P, D], F32, tag="gq_b")
            nc.gpsimd.dma_start(out=gq_b,
                                in_=gq_d[None, :].to_broadcast((P, D)))
            btq_b = consts.tile([P, D], F32, tag="btq_b")
            nc.gpsimd.dma_start(out=btq_b,
                                in_=btq_d[None, :].to_broadcast((P, D)))
            gkv_b = consts.tile([P, D], F32, tag="gkv_b")
            nc.gpsimd.dma_start(out=gkv_b,
                                in_=gkv_d[None, :].to_broadcast((P, D)))
            btkv_b = consts.tile([P, D], F32, tag="btkv_b")
            nc.gpsimd.dma_start(out=btkv_b,
                                in_=btkv_d[None, :].to_broadcast((P, D)))
        else:
            gq_b = btq_b = gkv_b = btkv_b = None
        if with_bias:
            bv_b = consts.tile([P, DG], F32, tag="bv_b")
            nc.gpsimd.dma_start(out=bv_b,
                                in_=bv_d[None, :].to_broadcast((P, DG)))
            bq_c = consts.tile([P, NM], F32, tag="bq_c")
            nc.sync.dma_start(out=bq_c,
                              in_=bq_d.rearrange("(m p) -> p m", p=P))
            bk_c = consts.tile([P, NM], F32, tag="bk_c")
            nc.sync.dma_start(out=bk_c,
                              in_=bk_d.rearrange("(m p) -> p m", p=P))
        else:
            bv_b = bq_c = bk_c = None

        nc.gpsimd.dma_start(out=wo_bf,
                            in_=wo.rearrange("(m p) n -> p m n", p=P))

        c_r = c_in.rearrange("(n i p) d -> n p i d", p=P, i=4)
        q_r = q_in.rearrange("(n i p) d -> n p i d", p=P, i=4)

        lncnt = [0]

        def ln_chunk(xb, g_b, b_b, lnpool, lntmp=None):
            """LN 4 token tiles xb[:, tl, :] -> list of [128, 1024] bf16."""
            lncnt[0] += 1
            mv = stats.tile([P, 4, 2], F32, tag="mv")
            for tl in range(4):
                st = stats.tile([P, 2, 6], F32, tag="bnst")
                nc.vector.bn_stats(out=st[:, 0, :], in_=xb[:, tl, 0:512])
                nc.vector.bn_stats(out=st[:, 1, :], in_=xb[:, tl, 512:1024])
                nc.vector.bn_aggr(out=mv[:, tl, :], in_=st)
            # rstd = exp(-0.5 * ln(var + eps)) -- stays in the exp table set
            lnv = stats.tile([P, 4], F32, tag="lnv")
            nc.scalar.activation(out=lnv, in_=mv[:, :, 1],
                                 func=mybir.ActivationFunctionType.Ln,
                                 bias=eps_t)
            rstd = stats.tile([P, 4], F32, tag="rstd")
            nc.scalar.activation(out=rstd, in_=lnv,
                                 func=mybir.ActivationFunctionType.Exp,
                                 scale=-0.5)
            lnts = []
            for tl in range(4):
                x = xb[:, tl, :]
                lnt = lnpool.tile([P, D], BF16, tag="ln",
                                  name=f"ln{tl}_{uq}_{lncnt[0]}")
                if not ln_affine:
                    nc.vector.tensor_scalar(
                        out=lnt, in0=x, scalar1=mv[:, tl, 0:1],
                        scalar2=rstd[:, tl:tl + 1],
                        op0=mybir.AluOpType.subtract,
                        op1=mybir.AluOpType.mult)
                else:
                    xc = lntmp.tile([P, D], F32, tag="xc")
                    nc.vector.tensor_scalar(
                        out=xc, in0=x, scalar1=mv[:, tl, 0:1],
                        scalar2=rstd[:, tl:tl + 1],
                        op0=mybir.AluOpType.subtract,
                        op1=mybir.AluOpType.mult)
                    xg = lntmp.tile([P, D], F32, tag="xg")
                    nc.vector.tensor_tensor(out=xg, in0=xc, in1=g_b,
                                            op=mybir.AluOpType.mult)
                    nc.vector.tensor_tensor(out=lnt, in0=xg, in1=b_b,
                                            op=mybir.AluOpType.add)
                lnts.append(lnt)
            return lnts

        def transpose_chunk(ln_tiles, ps_pool, lnT):
            """4 LN tiles ([128 tok, 1024 feat]) -> lnT [128 feat, 8, 512 tok]."""
            for s in range(KS):
                pt = ps_pool.tile([P, 512], BF16, tag="tr")
                for tl in range(4):
                    nc.tensor.transpose(pt[:, tl * P:(tl + 1) * P],
                                        ln_tiles[tl][:, s * P:(s + 1) * P],
                                        ident)
                nc.scalar.copy(out=lnT[:, s, :], in_=pt)
            return lnT

        # ========= phase 1: all LN / transposes / K,V proj / lnTq =========
        with (
            tc.tile_pool(name="wqkv", bufs=1) as wkvpool,
            tc.tile_pool(name="cx", bufs=3) as cxpool,
            tc.tile_pool(name="lnout", bufs=6) as lnpool1,
            tc.tile_pool(name="lnTc", bufs=2) as lntcpool,
            tc.tile_pool(name="lntmpc", bufs=2) as lntmpc,
            tc.tile_pool(name="ps_ctx", bufs=3, space="PSUM") as ps_ctx,
            tc.tile_pool(name="ps_trc", bufs=3, space="PSUM") as ps_trc,
        ):
            wq_bf = wkvpool.tile([P, KS, DG], BF16, tag="wq_bf")
            wk_bf = wkvpool.tile([P, KS, DG], BF16, tag="wk_bf")
            wv_bf = wkvpool.tile([P, KS, DG], BF16, tag="wv_bf")
            nc.sync.dma_start(out=wq_bf,
                              in_=wq.rearrange("(s p) n -> p s n", p=P))
            nc.gpsimd.dma_start(out=wk_bf,
                                in_=wk.rearrange("(s p) n -> p s n", p=P))
            nc.sync.dma_start(out=wv_bf,
                              in_=wv.rearrange("(s p) n -> p s n", p=P))

            for c in range(NCH):
                xb = cxpool.tile([P, 4, D], F32, tag="xb")
                (nc.sync if c % 2 == 0 else nc.gpsimd).dma_start(
                    out=xb, in_=c_r[c])
                ln_tiles = ln_chunk(xb, gkv_b, btkv_b, lnpool1, lntmpc)
                lnT = lntcpool.tile([P, KS, 512], BF16, tag="lnT")
                transpose_chunk(ln_tiles, ps_trc, lnT)
                for m in range(NM):
                    pp = ps_ctx.tile([P, 512], F32, tag="pp")
                    for s in range(KS):
                        nc.tensor.matmul(pp, lhsT=wk_bf[:, s, m * P:(m + 1) * P],
                                         rhs=lnT[:, s, :],
                                         start=(s == 0), stop=(s == KS - 1))
                    if with_bias:
                        nc.scalar.activation(
                            out=kt[c][:, m, :], in_=pp,
                            func=mybir.ActivationFunctionType.Identity,
                            bias=bk_c[:, m:m + 1])
                    else:
                        nc.scalar.copy(out=kt[c][:, m, :], in_=pp)
                for tl in range(4):
                    t = 4 * c + tl
                    pp = ps_ctx.tile([P, 512], F32, tag="pp")
                    for s in range(KS):
                        nc.tensor.matmul(pp, lhsT=lnT[:, s, tl * P:(tl + 1) * P],
                                         rhs=wv_bf[:, s, :],
                                         start=(s == 0), stop=(s == KS - 1))
                    if with_bias:
                        nc.vector.tensor_tensor(
                            out=vs[:, t, :, 0:HD],
                            in0=pp.rearrange("p (h d) -> p h d", h=NH),
                            in1=bv_b.rearrange("p (h d) -> p h d", h=NH),
                            op=mybir.AluOpType.add)
                    else:
                        nc.vector.tensor_copy(
                            out=vs[:, t, :, 0:HD],
                            in_=pp.rearrange("p (h d) -> p h d", h=NH))

            # query chunks: LN + transpose + Q^T proj
            for c in range(NCH):
                xb = cxpool.tile([P, 4, D], F32, tag="xb",
                                 name=f"qxb{c}_{uq}")
                (nc.sync if c % 2 == 0 else nc.gpsimd).dma_start(
                    out=xb, in_=q_r[c])
                ln_tiles = ln_chunk(xb, gq_b, btq_b, lnpool1, lntmpc)
                lnT = lntcpool.tile([P, KS, 512], BF16, tag="lnT",
                                    name=f"qlnT{c}_{uq}")
                transpose_chunk(ln_tiles, ps_trc, lnT)
                for m in range(NM):
                    pp = ps_ctx.tile([P, 512], F32, tag="pp",
                                     name=f"qpp{c}_{m}_{uq}")
                    for s in range(KS):
                        nc.tensor.matmul(pp, lhsT=wq_bf[:, s, m * P:(m + 1) * P],
                                         rhs=lnT[:, s, :],
                                         start=(s == 0), stop=(s == KS - 1))
                    if with_bias:
                        nc.vector.tensor_scalar(
                            out=qt[c][:, m, :], in0=pp,
                            scalar1=bq_c[:, m:m + 1], scalar2=None,
                            op0=mybir.AluOpType.add)
                    else:
                        nc.vector.tensor_copy(out=qt[c][:, m, :], in_=pp)

        # ================= phase 2: pure attention =================
        with (
            tc.tile_pool(name="exp", bufs=exp_bufs) as exppool,
            tc.tile_pool(name="smalls", bufs=2) as smalls,
            tc.tile_pool(name="yout", bufs=2) as ypool,
            tc.tile_pool(name="ps_s", bufs=2, space="PSUM") as ps_s,
            tc.tile_pool(name="ps_av", bufs=2, space="PSUM") as ps_av,
            tc.tile_pool(name="ps_wo", bufs=2, space="PSUM") as ps_wo,
        ):

            def emit_av_chunk(prev, kg):
                c0, j0, exp_pair, avs = prev
                for hl in range(2):
                    for k2 in range(2):
                        ki = kg * 2 + k2
                        nc.tensor.matmul(avs[hl], lhsT=vs[:, ki, 2 * j0 + hl, :],
                                         rhs=exp_pair[hl][:, ki, :],
                                         start=(ki == 0), stop=(ki == NT - 1),
                                         skip_group_check=True)

            def emit_normalize(prev):
                c0, j0, exp_pair, avs = prev
                for hl in range(2):
                    av = avs[hl]
                    zsb = smalls.tile([1, 512], F32, tag="zsb",
                                      name=f"zsb{c0}_{j0}_{hl}_{uq}")
                    nc.vector.tensor_copy(out=zsb, in_=av[HD:HD + 1, :])
                    zrow = smalls.tile([1, 512], F32, tag="zrow",
                                       name=f"zrow{c0}_{j0}_{hl}_{uq}")
                    nc.vector.reciprocal_approx_fast(out=zrow, in_=zsb)
                    rinv = smalls.tile([HD, 512], F32, tag="rinv",
                                       name=f"rinv{c0}_{j0}_{hl}_{uq}")
                    nc.gpsimd.partition_broadcast(rinv, zrow)
                    nc.vector.tensor_tensor(
                        out=os_t[c0][hl * HD:(hl + 1) * HD, j0, :],
                        in0=av[0:HD, :], in1=rinv,
                        op=mybir.AluOpType.mult)

            def emit_wo_group(c0, g):
                tl, dc = g // 2, g % 2
                t = 4 * c0 + tl
                pp = ps_wo.tile([P, 512], F32, tag="pw",
                               name=f"wopp{c0}_{g}_{uq}")
                for m in range(NM):
                    nc.tensor.matmul(
                        pp, lhsT=os_t[c0][:, m, tl * P:(tl + 1) * P],
                        rhs=wo_bf[:, m, dc * 512:(dc + 1) * 512],
                        start=(m == 0), stop=(m == NM - 1),
                        skip_group_check=True)
                yt = ypool.tile([P, 512], F32, tag="y",
                                name=f"yt{c0}_{g}_{uq}")
                nc.vector.tensor_copy(out=yt, in_=pp)
                nc.sync.dma_start(
                    out=y_out[t * P:(t + 1) * P, dc * 512:(dc + 1) * 512],
                    in_=yt)

            if _LV == 0:
                return

            prev = None
            wo_pending = []   # (c, next_group_idx, appended_it)
            for c in range(NCH):
                for j in range(NM):
                    it = c * NM + j
                    exp_pair = [exppool.tile([P, NT, 512], BF16, tag=f"exp{hl}",
                                             name=f"exp{hl}_{c}_{j}_{uq}")
                                for hl in range(2)]
                    for kg in range(8):
                        ps_pair = [ps_s.tile([P, 2, 512], F32, tag="psS",
                                             name=f"psS{hl}_{c}_{j}_{kg}_{uq}")
                                   for hl in range(2)]
                        for k2 in range(2):
                            ki = kg * 2 + k2
                            for hl in range(2):
                                rows = slice(hl * HD, (hl + 1) * HD)
                                nc.tensor.matmul(
                                    ps_pair[hl][:, k2, :],
                                    lhsT=kt[ki // 4][rows, j,
                                              (ki % 4) * P:(ki % 4 + 1) * P],
                                    rhs=qt[c][rows, j, :],
                                    start=True, stop=True,
                                    skip_group_check=True)
                        for hl in range(2):
                            if kg in (2, 5):
                                # Schraudolph-style exp on DVE: write the bf16
                                # bit pattern as an int16 value convert
                                nc.vector.tensor_scalar(
                                    out=exp_pair[hl][:, kg * 2:kg * 2 + 2, :]
                                        .bitcast(mybir.dt.int16),
                                    in0=ps_pair[hl][:, :, :],
                                    scalar1=ab_t[:, 0:1],
                                    scalar2=ab_t[:, 1:2],
                                    op0=mybir.AluOpType.mult,
                                    op1=mybir.AluOpType.add)
                            else:
                                nc.scalar.activation(
                                    out=exp_pair[hl][:, kg * 2:kg * 2 + 2, :],
                                    in_=ps_pair[hl][:, :, :],
                                    func=mybir.ActivationFunctionType.Exp,
                                    scale=SCALE)
                        if prev is not None and _LV >= 2 and kg < 4:
                            emit_av_chunk(prev, 2 * kg)
                            emit_av_chunk(prev, 2 * kg + 1)
                        # Wo groups: only once the pending chunk's normalize has
                        # had >= a full iteration of PE runway (kg7 of it+1).
                        if _LV >= 4 and wo_pending and kg in (3, 7):
                            c0, g, ait = wo_pending[0]
                            if it > ait + 1 or (it == ait + 1 and kg == 7):
                                emit_wo_group(c0, g)
                                if g + 1 >= 8:
                                    wo_pending.pop(0)
                                else:
                                    wo_pending[0] = (c0, g + 1, ait)
                    if prev is not None and _LV >= 3:
                        emit_normalize(prev)
                        if prev[1] == NM - 1:      # finished batch-chunk prev[0]
                            wo_pending.append((prev[0], 0, it))
                    avs = [ps_av.tile([HD + 1, 512], F32, tag="av",
                                      name=f"av{c}_{j}_{hl}_{uq}")
                           for hl in range(2)]
                    prev = (c, j, exp_pair, avs)
            # drain: AV + normalize of the last (c,j), then remaining Wo groups
            if _LV >= 2:
                for kg in range(8):
                    emit_av_chunk(prev, kg)
            if _LV >= 3:
                emit_normalize(prev)
            if _LV >= 4:
                wo_pending.append((prev[0], 0, 0))
                for c0, g0, _ait in list(wo_pending):
                    for g in range(g0, 8):
                        emit_wo_group(c0, g)


_CACHE = {}


def _get_exec(ln_affine=True, with_bias=True, repeat=1, hw_loop=0,
              probe="full"):
    """Build the Bass program once and wrap it in a reusable jitted executor."""
    key = ("exec", ln_affine, with_bias, repeat, hw_loop, probe)
    if key in _CACHE:
        return _CACHE[key]

    import jax
    from jax.sharding import Mesh, PartitionSpec
    from jax.experimental.shard_map import shard_map
    from concourse import bass2jax

    nc = _build_program(ln_affine=ln_affine, with_bias=with_bias,
                        repeat=repeat, hw_loop=hw_loop, probe=probe)
    bass2jax.install_neuronx_cc_hook()

    partition_name = (nc.partition_id_tensor.name
                      if nc.partition_id_tensor else None)
    in_names, out_names, out_avals, zero_shapes = [], [], [], []
    in_dtypes = {}
    for alloc in nc.m.functions[0].allocations:
        if not isinstance(alloc, mybir.MemoryLocationSet):
            continue
        name = alloc.memorylocations[0].name
        if alloc.kind == "ExternalInput":
            if name != partition_name:
                in_names.append(name)
                in_dtypes[name] = mybir.dt.np(alloc.dtype)
        elif alloc.kind == "ExternalOutput":
            shape = tuple(alloc.tensor_shape)
            dtype = mybir.dt.np(alloc.dtype)
            out_names.append(name)
            out_avals.append(jax.core.ShapedArray(shape, dtype))
            zero_shapes.append((shape, dtype))
    n_params = len(in_names)
    n_outs = len(out_avals)
    all_names = list(in_names) + list(out_names)
    if partition_name is not None:
        all_names.append(partition_name)
    donate = tuple(range(n_params, n_params + n_outs))

    def _body(*args):
        operands = list(args)
        if partition_name is not None:
            operands.append(bass2jax.partition_id_tensor())
        outs = bass2jax._bass_exec_p.bind(
            *operands,
            out_avals=tuple(out_avals),
            in_names=tuple(all_names),
            out_names=tuple(out_names),
            lowering_input_output_aliases=(),
            sim_require_finite=True,
            sim_require_nnan=True,
            nc=nc,
        )
        return tuple(outs)

    n_cores = 8
    devices = jax.devices()[:n_cores]
    mesh = Mesh(np.asarray(devices), ("core",))
    in_specs = (PartitionSpec("core"),) * (n_params + n_outs)
    out_specs = (PartitionSpec("core"),) * n_outs
    sharded = jax.jit(
        shard_map(_body, mesh=mesh, in_specs=in_specs, out_specs=out_specs,
                  check_rep=False),
        donate_argnums=donate, keep_unused=True)

    def execute(in_maps):
        per_core = [[np.ascontiguousarray(np.asarray(m[name], in_dtypes[name]))
                     for name in in_names] for m in in_maps]
        concat_in = [np.concatenate([per_core[cc][i] for cc in range(n_cores)],
                                    axis=0) for i in range(n_params)]
        concat_zeros = [np.zeros((n_cores * s[0], *s[1:]), d)
                        for (s, d) in zero_shapes]
        out_arrs = sharded(*concat_in, *concat_zeros)
        return [
            {name: np.asarray(out_arrs[i]).reshape(n_cores, *out_avals[i].shape)[cc]
             for i, name in enumerate(out_names)}
            for cc in range(n_cores)
        ]

    _CACHE[key] = execute
    _CACHE[("parts", ln_affine, with_bias, repeat, hw_loop, probe)] = {
        "sharded": sharded, "in_names": in_names, "in_dtypes": in_dtypes,
        "n_params": n_params,
        "out_names": out_names, "out_avals": out_avals,
        "zero_shapes": zero_shapes, "mesh": mesh, "n_cores": n_cores,
        "body": _body, "in_specs": in_specs, "out_specs": out_specs,
        "donate": donate,
    }
    return execute


def _time_exec(in_maps, iters=5, ln_affine=True, with_bias=True,
               repeat=1, hw_loop=0, probe="full"):
    """Time the sharded executable with device-resident inputs (seconds)."""
    import time
    import jax
    from jax.sharding import NamedSharding, PartitionSpec

    _get_exec(ln_affine=ln_affine, with_bias=with_bias, repeat=repeat,
              hw_loop=hw_loop, probe=probe)
    parts = _CACHE[("parts", ln_affine, with_bias, repeat, hw_loop, probe)]
    sharded = parts["sharded"]
    n_cores = parts["n_cores"]
    in_dtypes = parts["in_dtypes"]
    sh = NamedSharding(parts["mesh"], PartitionSpec("core"))
    per_core = [[np.ascontiguousarray(np.asarray(m[name], in_dtypes[name]))
                 for name in parts["in_names"]] for m in in_maps]
    concat_in = [np.concatenate([per_core[cc][i] for cc in range(n_cores)],
                                axis=0) for i in range(parts["n_params"])]
    in_dev = [jax.device_put(a, sh) for a in concat_in]
    jax.block_until_ready(in_dev)
    times = []
    for _ in range(iters):
        z_dev = [jax.device_put(
                     np.zeros((n_cores * s[0], *s[1:]), d), sh)
                 for (s, d) in parts["zero_shapes"]]
        jax.block_until_ready(z_dev)
        t0 = time.perf_counter()
        out = sharded(*in_dev, *z_dev)
        jax.block_until_ready(out)
        times.append(time.perf_counter() - t0)
        del out
    return times


def _ln_is_identity(inputs):
    return all(
        np.all(np.asarray(inputs[k], np.float32) == v)
        for k, v in (("gq", 1.0), ("betq", 0.0), ("gkv", 1.0), ("betkv", 0.0))
    )


def _bias_is_zero(inputs):
    return all(
        np.all(np.asarray(inputs[k], np.float32) == 0.0)
        for k in ("bq", "bk", "bv")
    )


def _make_in_maps(inputs):
    import ml_dtypes
    BF = ml_dtypes.bfloat16
    q = np.asarray(inputs["query"], np.float32)
    c = np.asarray(inputs["context"], np.float32)
    Wq = np.asarray(inputs["Wq"], np.float32).astype(BF)
    Wk = np.asarray(inputs["Wk"], np.float32).astype(BF)
    Wv = np.asarray(inputs["Wv"], np.float32).astype(BF)
    Wo = np.asarray(inputs["Wo"], np.float32).astype(BF)
    bq = np.asarray(inputs["bq"], np.float32)
    bk = np.asarray(inputs["bk"], np.float32)
    bv = np.asarray(inputs["bv"], np.float32)
    gq = np.asarray(inputs["gq"], np.float32)
    btq = np.asarray(inputs["betq"], np.float32)
    gkv = np.asarray(inputs["gkv"], np.float32)
    btkv = np.asarray(inputs["betkv"], np.float32)
    ln_affine = not _ln_is_identity(inputs)
    with_bias = not _bias_is_zero(inputs)
    in_maps = []
    for core in range(8):
        b, hg = core // 2, core % 2
        sl = slice(hg * DG, (hg + 1) * DG)
        m = {
            "q_in": q[b], "c_in": c[b],
            "wq": Wq[:, sl], "wk": Wk[:, sl], "wv": Wv[:, sl],
            "wo": Wo[sl, :],
        }
        if with_bias:
            m.update({"bq": bq[sl], "bk": bk[sl], "bv": bv[sl]})
        if ln_affine:
            m.update({"gq": gq, "btq": btq, "gkv": gkv, "btkv": btkv})
        in_maps.append(m)
    return in_maps


def kernel(**inputs):
    ln_affine = not _ln_is_identity(inputs)
    with_bias = not _bias_is_zero(inputs)
    execute = _get_exec(ln_affine=ln_affine, with_bias=with_bias)
    in_maps = _make_in_maps(inputs)
    results = execute(in_maps)
    bo = np.asarray(inputs["bo"], np.float32)
    B = 4
    out = np.empty((B, N_TOK, D), np.float32)
    for b in range(B):
        out[b] = results[2 * b]["y_out"] + results[2 * b + 1]["y_out"] + bo
    return out


# revision 24
# speedup vs baseline: 1.0342x; 1.0022x over previous
"""Trainium2 Bass kernel for nn_CrossAttention (B=4, Nq=Nk=2048, D=1024, H=16).

Sharding: 8 cores = (batch b in 0..3) x (head-group hg in 0..1), 8 heads/core.
Each core gets its batch's query/context plus the column slice of Wq/Wk/Wv and
row slice of Wo for its 8 heads; LayerNorm params are replicated.  Host sums
the two head-group partial outputs per batch and adds bo.

Per-core pipeline (all matmuls bf16 with fp32 PSUM accumulation):
  Phase 1 (context): LN (fp32, bn_stats; rstd = exp(-0.5*ln(var+eps)) so the
  whole kernel uses ONE activation table set - no table-switch stalls)
  -> PE transpose -> K^T / V projections.  PSUM->SBUF moves ride ScalarE /
  DVE (ScalarE is otherwise idle here).
  Phase 2 (query proj + attention, software-pipelined): the q-chunk LN /
  transpose / Q^T projection for chunk c+1 is emitted inside the attention
  loop over chunk c, so DVE/PE/Pool chew projection work while ScalarE
  streams exp.  ScalarE does NOTHING but exp in this phase (the Q^T
  PSUM->SBUF moves ride the Pool engine).
  S^T = K Q^T tiles (2 heads packed in the 128-row PE array via auto
  tile_position row tiling -> concurrent matmul pairs on HW)
  -> exp on ScalarE with the 1/sqrt(dh) scale folded in
  -> AV matmul with M=65 (row 64 = softmax denominator Z, for free)
  -> normalize via reciprocal_approx_fast straight from PSUM + GPSIMD
  partition_broadcast + DVE -> Wo row-slice matmul (deferred half an
  iteration so the PE queue head never blocks on the normalize chain)
  -> fp32 partial output.
"""

import numpy as np

import concourse.bass as bass
import concourse.mybir as mybir
import concourse.tile as tile
from concourse import bacc
from concourse.masks import make_identity

P = 128
N_TOK = 2048          # tokens per batch (both Nq and Nk)
D = 1024              # model dim
KS = D // P           # 8 contraction subtiles
DG = 512              # per-core projection width (8 heads * 64)
NM = DG // P          # 4 output blocks / head-pair groups
NH = 8                # heads per core
HD = 64
NT = N_TOK // P       # 16 token tiles
NCH = N_TOK // 512    # 4 token chunks of 512
SCALE = HD ** -0.5
EPS = 1e-5

F32 = mybir.dt.float32
BF16 = mybir.dt.bfloat16
_UNIQ = [0]


def _build_program(ln_affine=True, with_bias=True, repeat=1, hw_loop=0,
                   probe="full"):
    nc = bacc.Bacc("TRN2", target_bir_lowering=False, debug=False)

    q_in = nc.dram_tensor("q_in", (N_TOK, D), F32, kind="ExternalInput")
    c_in = nc.dram_tensor("c_in", (N_TOK, D), F32, kind="ExternalInput")
    wq = nc.dram_tensor("wq", (D, DG), BF16, kind="ExternalInput")
    wk = nc.dram_tensor("wk", (D, DG), BF16, kind="ExternalInput")
    wv = nc.dram_tensor("wv", (D, DG), BF16, kind="ExternalInput")
    wo = nc.dram_tensor("wo", (DG, D), BF16, kind="ExternalInput")
    if with_bias:
        bq_d = nc.dram_tensor("bq", (DG,), F32, kind="ExternalInput")
        bk_d = nc.dram_tensor("bk", (DG,), F32, kind="ExternalInput")
        bv_d = nc.dram_tensor("bv", (DG,), F32, kind="ExternalInput")
    else:
        bq_d = bk_d = bv_d = None
    if ln_affine:
        gq_d = nc.dram_tensor("gq", (D,), F32, kind="ExternalInput")
        btq_d = nc.dram_tensor("btq", (D,), F32, kind="ExternalInput")
        gkv_d = nc.dram_tensor("gkv", (D,), F32, kind="ExternalInput")
        btkv_d = nc.dram_tensor("btkv", (D,), F32, kind="ExternalInput")
    else:
        gq_d = btq_d = gkv_d = btkv_d = None
    y_out = nc.dram_tensor("y_out", (N_TOK, D), F32, kind="ExternalOutput")

    import contextlib

    with tile.TileContext(nc) as tc:
        loop_ctx = tc.For_i(0, hw_loop, 1) if hw_loop else None
        with (loop_ctx if loop_ctx is not None else contextlib.nullcontext()):
         for _rep in range(repeat):
            _UNIQ[0] += 1
            _emit_kernel(nc, tc, q_in, c_in, wq, wk, wv, wo,
                         bq_d, bk_d, bv_d, gq_d, btq_d, gkv_d, btkv_d,
                         y_out, ln_affine, with_bias, probe)

    nc.finalize()
    return nc


def _emit_kernel(nc, tc, q_in, c_in, wq, wk, wv, wo,
                 bq_d, bk_d, bv_d, gq_d, btq_d, gkv_d, btkv_d,
                 y_out, ln_affine, with_bias, probe="full"):
    # probe: "ctx" (phase 1 + qproj only), "sexp" (+ S/exp), "av" (+ AV),
    #        "norm" (+ normalize), "full"
    _LV = {"ctx": 0, "sexp": 1, "av": 2, "norm": 3, "full": 4}[probe]
    uq = _UNIQ[0]
    exp_bufs = 1 if ln_affine else 3
    with (
        tc.tile_pool(name="persist", bufs=1) as persist,
        tc.tile_pool(name="wqo", bufs=1) as wqo,
        tc.tile_pool(name="consts", bufs=1) as consts,
        tc.tile_pool(name="stats", bufs=4) as stats,
    ):
        # ---------------- persistent tensors ----------------
        qt = [persist.tile([P, NM, 512], BF16, tag=f"qt{c}", name=f"qt{c}_{uq}")
              for c in range(NCH)]   # Q^T per token chunk
        kt = [persist.tile([P, NM, 512], BF16, tag=f"kt{c}", name=f"kt{c}_{uq}")
              for c in range(NCH)]   # K^T per key chunk
        vs = persist.tile([P, NT, NH, HD + 1], BF16, tag="vs")
        os_t = [persist.tile([P, NM, 512], BF16, tag=f"os{c}",
                             name=f"os{c}_{uq}")
                for c in range(NCH)]
        nc.vector.memset(vs[:, :, :, HD:HD + 1], 1.0)
        wo_bf = wqo.tile([P, NM, D], BF16, tag="wo_bf")

        # ---------------- constants ----------------
        ident = consts.tile([P, P], BF16, tag="ident")
        make_identity(nc, ident)
        eps_t = consts.tile([P, 1], F32, tag="eps")
        nc.vector.memset(eps_t, EPS)
        # DVE bit-trick exp constants: bf16bits(exp(s*SCALE)) ~ int16(A*s + B)
        ab_t = consts.tile([P, 2], F32, tag="ab")
        nc.vector.memset(ab_t[:, 0:1], 128.0 * 1.4426950408889634 * SCALE)
        nc.vector.memset(ab_t[:, 1:2], 127.0 * 128.0 - 7.0)
        if ln_affine:
            gq_b = consts.tile([P, D], F32, tag="gq_b")
            nc.gpsimd.dma_start(out=gq_b,
                                in_=gq_d[None, :].to_broadcast((P, D)))
            btq_b = consts.tile([P, D], F32, tag="btq_b")
            nc.gpsimd.dma_start(out=btq_b,
                                in_=btq_d[None, :].to_broadcast((P, D)))
            gkv_b = consts.tile([P, D], F32, tag="gkv_b")
            nc.gpsimd.dma_start(out=gkv_b,
                                in_=gkv_d[None, :].to_broadcast((P, D)))
            btkv_b = consts.tile([P, D], F32, tag="btkv_b")
            nc.gpsimd.dma_start(out=btkv_b,
                                in_=btkv_d[None, :].to_broadcast((P, D)))
        else:
            gq_b = btq_b = gkv_b = btkv_b = None
        if with_bias:
            bv_b = consts.tile([P, DG], F32, tag="bv_b")
            nc.gpsimd.dma_start(out=bv_b,
                                in_=bv_d[None, :].to_broadcast((P, DG)))
            bq_c = consts.tile([P, NM], F32, tag="bq_c")
            nc.sync.dma_start(out=bq_c,
                              in_=bq_d.rearrange("(m p) -> p m", p=P))
            bk_c = consts.tile([P, NM], F32, tag="bk_c")
            nc.sync.dma_start(out=bk_c,
                              in_=bk_d.rearrange("(m p) -> p m", p=P))
        else:
            bv_b = bq_c = bk_c = None

        nc.gpsimd.dma_start(out=wo_bf,
                            in_=wo.rearrange("(m p) n -> p m n", p=P))

        c_r = c_in.rearrange("(n i p) d -> n p i d", p=P, i=4)
        q_r = q_in.rearrange("(n i p) d -> n p i d", p=P, i=4)

        lncnt = [0]

        def ln_chunk(xb, g_b, b_b, lnpool, lntmp=None):
            """LN 4 token tiles xb[:, tl, :] -> list of [128, 1024] bf16."""
            lncnt[0] += 1
            mv = stats.tile([P, 4, 2], F32, tag="mv")
            for tl in range(4):
                st = stats.tile([P, 2, 6], F32, tag="bnst")
                nc.vector.bn_stats(out=st[:, 0, :], in_=xb[:, tl, 0:512])
                nc.vector.bn_stats(out=st[:, 1, :], in_=xb[:, tl, 512:1024])
                nc.vector.bn_aggr(out=mv[:, tl, :], in_=st)
            # rstd = exp(-0.5 * ln(var + eps)) -- stays in the exp table set
            lnv = stats.tile([P, 4], F32, tag="lnv")
            nc.scalar.activation(out=lnv, in_=mv[:, :, 1],
                                 func=mybir.ActivationFunctionType.Ln,
                                 bias=eps_t)
            rstd = stats.tile([P, 4], F32, tag="rstd")
            nc.scalar.activation(out=rstd, in_=lnv,
                                 func=mybir.ActivationFunctionType.Exp,
                                 scale=-0.5)
            lnts = []
            for tl in range(4):
                x = xb[:, tl, :]
                lnt = lnpool.tile([P, D], BF16, tag="ln",
                                  name=f"ln{tl}_{uq}_{lncnt[0]}")
                if not ln_affine:
                    nc.vector.tensor_scalar(
                        out=lnt, in0=x, scalar1=mv[:, tl, 0:1],
                        scalar2=rstd[:, tl:tl + 1],
                        op0=mybir.AluOpType.subtract,
                        op1=mybir.AluOpType.mult)
                else:
                    xc = lntmp.tile([P, D], F32, tag="xc")
                    nc.vector.tensor_scalar(
                        out=xc, in0=x, scalar1=mv[:, tl, 0:1],
                        scalar2=rstd[:, tl:tl + 1],
                        op0=mybir.AluOpType.subtract,
                        op1=mybir.AluOpType.mult)
                    xg = lntmp.tile([P, D], F32, tag="xg")
                    nc.vector.tensor_tensor(out=xg, in0=xc, in1=g_b,
                                            op=mybir.AluOpType.mult)
                    nc.vector.tensor_tensor(out=lnt, in0=xg, in1=b_b,
                                            op=mybir.AluOpType.add)
                lnts.append(lnt)
            return lnts

        def transpose_chunk(ln_tiles, ps_pool, lnT):
            """4 LN tiles ([128 tok, 1024 feat]) -> lnT [128 feat, 8, 512 tok]."""
            for s in range(KS):
                pt = ps_pool.tile([P, 512], BF16, tag="tr")
                for tl in range(4):
                    nc.tensor.transpose(pt[:, tl * P:(tl + 1) * P],
                                        ln_tiles[tl][:, s * P:(s + 1) * P],
                                        ident)
                if s % 2 == 0:
                    nc.scalar.copy(out=lnT[:, s, :], in_=pt)
                else:
                    nc.vector.tensor_copy(out=lnT[:, s, :], in_=pt)
            return lnT

        # ========= phase 1: all LN / transposes / K,V proj / lnTq =========
        with (
            tc.tile_pool(name="wqkv", bufs=1) as wkvpool,
            tc.tile_pool(name="cx", bufs=3) as cxpool,
            tc.tile_pool(name="lnout", bufs=6) as lnpool1,
            tc.tile_pool(name="lnTc", bufs=2) as lntcpool,
            tc.tile_pool(name="lntmpc", bufs=2) as lntmpc,
            tc.tile_pool(name="ps_ctx", bufs=3, space="PSUM") as ps_ctx,
            tc.tile_pool(name="ps_trc", bufs=3, space="PSUM") as ps_trc,
        ):
            wq_bf = wkvpool.tile([P, KS, DG], BF16, tag="wq_bf")
            wk_bf = wkvpool.tile([P, KS, DG], BF16, tag="wk_bf")
            wv_bf = wkvpool.tile([P, KS, DG], BF16, tag="wv_bf")
            nc.sync.dma_start(out=wq_bf,
                              in_=wq.rearrange("(s p) n -> p s n", p=P))
            nc.gpsimd.dma_start(out=wk_bf,
                                in_=wk.rearrange("(s p) n -> p s n", p=P))
            nc.sync.dma_start(out=wv_bf,
                              in_=wv.rearrange("(s p) n -> p s n", p=P))

            def ctx_chunk(c):
                xb = cxpool.tile([P, 4, D], F32, tag="xb")
                (nc.sync if c % 2 == 0 else nc.gpsimd).dma_start(
                    out=xb, in_=c_r[c])
                ln_tiles = ln_chunk(xb, gkv_b, btkv_b, lnpool1, lntmpc)
                lnT = lntcpool.tile([P, KS, 512], BF16, tag="lnT")
                transpose_chunk(ln_tiles, ps_trc, lnT)
                for m in range(NM):
                    pp = ps_ctx.tile([P, 512], F32, tag="pp")
                    for s in range(KS):
                        nc.tensor.matmul(pp, lhsT=wk_bf[:, s, m * P:(m + 1) * P],
                                         rhs=lnT[:, s, :],
                                         start=(s == 0), stop=(s == KS - 1))
                    if with_bias:
                        nc.scalar.activation(
                            out=kt[c][:, m, :], in_=pp,
                            func=mybir.ActivationFunctionType.Identity,
                            bias=bk_c[:, m:m + 1])
                    else:
                        nc.scalar.copy(out=kt[c][:, m, :], in_=pp)
                for tl in range(4):
                    t = 4 * c + tl
                    pp = ps_ctx.tile([P, 512], F32, tag="pp")
                    for s in range(KS):
                        nc.tensor.matmul(pp, lhsT=lnT[:, s, tl * P:(tl + 1) * P],
                                         rhs=wv_bf[:, s, :],
                                         start=(s == 0), stop=(s == KS - 1))
                    if with_bias:
                        nc.vector.tensor_tensor(
                            out=vs[:, t, :, 0:HD],
                            in0=pp.rearrange("p (h d) -> p h d", h=NH),
                            in1=bv_b.rearrange("p (h d) -> p h d", h=NH),
                            op=mybir.AluOpType.add)
                    else:
                        nc.vector.tensor_copy(
                            out=vs[:, t, :, 0:HD],
                            in_=pp.rearrange("p (h d) -> p h d", h=NH))

            # query chunks: LN + transpose + Q^T proj
            def q_chunk(c):
                xb = cxpool.tile([P, 4, D], F32, tag="xb",
                                 name=f"qxb{c}_{uq}")
                (nc.gpsimd if c % 2 == 0 else nc.sync).dma_start(
                    out=xb, in_=q_r[c])
                ln_tiles = ln_chunk(xb, gq_b, btq_b, lnpool1, lntmpc)
                lnT = lntcpool.tile([P, KS, 512], BF16, tag="lnT",
                                    name=f"qlnT{c}_{uq}")
                transpose_chunk(ln_tiles, ps_trc, lnT)
                for m in range(NM):
                    pp = ps_ctx.tile([P, 512], F32, tag="pp",
                                     name=f"qpp{c}_{m}_{uq}")
                    for s in range(KS):
                        nc.tensor.matmul(pp, lhsT=wq_bf[:, s, m * P:(m + 1) * P],
                                         rhs=lnT[:, s, :],
                                         start=(s == 0), stop=(s == KS - 1))
                    if with_bias:
                        nc.vector.tensor_scalar(
                            out=qt[c][:, m, :], in0=pp,
                            scalar1=bq_c[:, m:m + 1], scalar2=None,
                            op0=mybir.AluOpType.add)
                    else:
                        nc.vector.tensor_copy(out=qt[c][:, m, :], in_=pp)

            for c in range(NCH):
                ctx_chunk(c)
                q_chunk(c)

        # ================= phase 2: pure attention =================
        with (
            tc.tile_pool(name="exp", bufs=exp_bufs) as exppool,
            tc.tile_pool(name="smalls", bufs=2) as smalls,
            tc.tile_pool(name="yout", bufs=2) as ypool,
            tc.tile_pool(name="ps_s", bufs=2, space="PSUM") as ps_s,
            tc.tile_pool(name="ps_av", bufs=2, space="PSUM") as ps_av,
            tc.tile_pool(name="ps_wo", bufs=2, space="PSUM") as ps_wo,
        ):

            def emit_av_chunk(prev, kg):
                c0, j0, exp_pair, avs = prev
                for hl in range(2):
                    for k2 in range(2):
                        ki = kg * 2 + k2
                        nc.tensor.matmul(avs[hl], lhsT=vs[:, ki, 2 * j0 + hl, :],
                                         rhs=exp_pair[hl][:, ki, :],
                                         start=(ki == 0), stop=(ki == NT - 1),
                                         skip_group_check=True)

            def emit_normalize(prev):
                c0, j0, exp_pair, avs = prev
                for hl in range(2):
                    av = avs[hl]
                    zsb = smalls.tile([1, 512], F32, tag="zsb",
                                      name=f"zsb{c0}_{j0}_{hl}_{uq}")
                    nc.vector.tensor_copy(out=zsb, in_=av[HD:HD + 1, :])
                    zrow = smalls.tile([1, 512], F32, tag="zrow",
                                       name=f"zrow{c0}_{j0}_{hl}_{uq}")
                    nc.vector.reciprocal_approx_fast(out=zrow, in_=zsb)
                    rinv = smalls.tile([HD, 512], F32, tag="rinv",
                                       name=f"rinv{c0}_{j0}_{hl}_{uq}")
                    nc.gpsimd.partition_broadcast(rinv, zrow)
                    nc.vector.tensor_tensor(
                        out=os_t[c0][hl * HD:(hl + 1) * HD, j0, :],
                        in0=av[0:HD, :], in1=rinv,
                        op=mybir.AluOpType.mult)

            def emit_wo_group(c0, g):
                tl, dc = g // 2, g % 2
                t = 4 * c0 + tl
                pp = ps_wo.tile([P, 512], F32, tag="pw",
                               name=f"wopp{c0}_{g}_{uq}")
                for m in range(NM):
                    nc.tensor.matmul(
                        pp, lhsT=os_t[c0][:, m, tl * P:(tl + 1) * P],
                        rhs=wo_bf[:, m, dc * 512:(dc + 1) * 512],
                        start=(m == 0), stop=(m == NM - 1),
                        skip_group_check=True)
                yt = ypool.tile([P, 512], F32, tag="y",
                                name=f"yt{c0}_{g}_{uq}")
                nc.vector.tensor_copy(out=yt, in_=pp)
                nc.sync.dma_start(
                    out=y_out[t * P:(t + 1) * P, dc * 512:(dc + 1) * 512],
                    in_=yt)

            if _LV == 0:
                return

            prev = None
            wo_pending = []   # (c, next_group_idx, appended_it)
            for c in range(NCH):
                for j in range(NM):
                    it = c * NM + j
                    exp_pair = [exppool.tile([P, NT, 512], BF16, tag=f"exp{hl}",
                                             name=f"exp{hl}_{c}_{j}_{uq}")
                                for hl in range(2)]
                    for kg in range(8):
                        ps_pair = [ps_s.tile([P, 2, 512], F32, tag="psS",
                                             name=f"psS{hl}_{c}_{j}_{kg}_{uq}")
                                   for hl in range(2)]
                        for k2 in range(2):
                            ki = kg * 2 + k2
                            for hl in range(2):
                                rows = slice(hl * HD, (hl + 1) * HD)
                                nc.tensor.matmul(
                                    ps_pair[hl][:, k2, :],
                                    lhsT=kt[ki // 4][rows, j,
                                              (ki % 4) * P:(ki % 4 + 1) * P],
                                    rhs=qt[c][rows, j, :],
                                    start=True, stop=True,
                                    skip_group_check=True)
                        for hl in range(2):
                            if kg in (2, 5):
                                # Schraudolph-style exp on DVE: write the bf16
                                # bit pattern as an int16 value convert
                                nc.vector.tensor_scalar(
                                    out=exp_pair[hl][:, kg * 2:kg * 2 + 2, :]
                                        .bitcast(mybir.dt.int16),
                                    in0=ps_pair[hl][:, :, :],
                                    scalar1=ab_t[:, 0:1],
                                    scalar2=ab_t[:, 1:2],
                                    op0=mybir.AluOpType.mult,
                                    op1=mybir.AluOpType.add)
                            else:
                                nc.scalar.activation(
                                    out=exp_pair[hl][:, kg * 2:kg * 2 + 2, :],
                                    in_=ps_pair[hl][:, :, :],
                                    func=mybir.ActivationFunctionType.Exp,
                                    scale=SCALE)
                        if prev is not None and _LV >= 2 and kg < 4:
                            emit_av_chunk(prev, 2 * kg)
                            emit_av_chunk(prev, 2 * kg + 1)
                        # Wo groups: only once the pending chunk's normalize has
                        # had >= a full iteration of PE runway (kg7 of it+1).
                        if _LV >= 4 and wo_pending and kg in (3, 7):
                            c0, g, ait = wo_pending[0]
                            if it > ait + 1 or (it == ait + 1 and kg == 7):
                                emit_wo_group(c0, g)
                                if g + 1 >= 8:
                                    wo_pending.pop(0)
                                else:
                                    wo_pending[0] = (c0, g + 1, ait)
                    if prev is not None and _LV >= 3:
                        emit_normalize(prev)
                        if prev[1] == NM - 1:      # finished batch-chunk prev[0]
                            wo_pending.append((prev[0], 0, it))
                    avs = [ps_av.tile([HD + 1, 512], F32, tag="av",
                                      name=f"av{c}_{j}_{hl}_{uq}")
                           for hl in range(2)]
                    prev = (c, j, exp_pair, avs)
            # drain: AV + normalize of the last (c,j), then remaining Wo groups
            if _LV >= 2:
                for kg in range(8):
                    emit_av_chunk(prev, kg)
            if _LV >= 3:
                emit_normalize(prev)
            if _LV >= 4:
                wo_pending.append((prev[0], 0, 0))
                for c0, g0, _ait in list(wo_pending):
                    for g in range(g0, 8):
                        emit_wo_group(c0, g)


_CACHE = {}


def _get_exec(ln_affine=True, with_bias=True, repeat=1, hw_loop=0,
              probe="full"):
    """Build the Bass program once and wrap it in a reusable jitted executor."""
    key = ("exec", ln_affine, with_bias, repeat, hw_loop, probe)
    if key in _CACHE:
        return _CACHE[key]

    import jax
    from jax.sharding import Mesh, PartitionSpec
    from jax.experimental.shard_map import shard_map
    from concourse import bass2jax

    nc = _build_program(ln_affine=ln_affine, with_bias=with_bias,
                        repeat=repeat, hw_loop=hw_loop, probe=probe)
    bass2jax.install_neuronx_cc_hook()

    partition_name = (nc.partition_id_tensor.name
                      if nc.partition_id_tensor else None)
    in_names, out_names, out_avals, zero_shapes = [], [], [], []
    in_dtypes = {}
    for alloc in nc.m.functions[0].allocations:
        if not isinstance(alloc, mybir.MemoryLocationSet):
            continue
        name = alloc.memorylocations[0].name
        if alloc.kind == "ExternalInput":
            if name != partition_name:
                in_names.append(name)
                in_dtypes[name] = mybir.dt.np(alloc.dtype)
        elif alloc.kind == "ExternalOutput":
            shape = tuple(alloc.tensor_shape)
            dtype = mybir.dt.np(alloc.dtype)
            out_names.append(name)
            out_avals.append(jax.core.ShapedArray(shape, dtype))
            zero_shapes.append((shape, dtype))
    n_params = len(in_names)
    n_outs = len(out_avals)
    all_names = list(in_names) + list(out_names)
    if partition_name is not None:
        all_names.append(partition_name)
    donate = tuple(range(n_params, n_params + n_outs))

    def _body(*args):
        operands = list(args)
        if partition_name is not None:
            operands.append(bass2jax.partition_id_tensor())
        outs = bass2jax._bass_exec_p.bind(
            *operands,
            out_avals=tuple(out_avals),
            in_names=tuple(all_names),
            out_names=tuple(out_names),
            lowering_input_output_aliases=(),
            sim_require_finite=True,
            sim_require_nnan=True,
            nc=nc,
        )
        return tuple(outs)

    n_cores = 8
    devices = jax.devices()[:n_cores]
    mesh = Mesh(np.asarray(devices), ("core",))
    in_specs = (PartitionSpec("core"),) * (n_params + n_outs)
    out_specs = (PartitionSpec("core"),) * n_outs
    sharded = jax.jit(
        shard_map(_body, mesh=mesh, in_specs=in_specs, out_specs=out_specs,
                  check_rep=False),
        donate_argnums=donate, keep_unused=True)

    def execute(in_maps):
        per_core = [[np.ascontiguousarray(np.asarray(m[name], in_dtypes[name]))
                     for name in in_names] for m in in_maps]
        concat_in = [np.concatenate([per_core[cc][i] for cc in range(n_cores)],
                                    axis=0) for i in range(n_params)]
        concat_zeros = [np.zeros((n_cores * s[0], *s[1:]), d)
                        for (s, d) in zero_shapes]
        out_arrs = sharded(*concat_in, *concat_zeros)
        return [
            {name: np.asarray(out_arrs[i]).reshape(n_cores, *out_avals[i].shape)[cc]
             for i, name in enumerate(out_names)}
            for cc in range(n_cores)
        ]

    _CACHE[key] = execute
    _CACHE[("parts", ln_affine, with_bias, repeat, hw_loop, probe)] = {
        "sharded": sharded, "in_names": in_names, "in_dtypes": in_dtypes,
        "n_params": n_params,
        "out_names": out_names, "out_avals": out_avals,
        "zero_shapes": zero_shapes, "mesh": mesh, "n_cores": n_cores,
        "body": _body, "in_specs": in_specs, "out_specs": out_specs,
        "donate": donate,
    }
    return execute


def _time_exec(in_maps, iters=5, ln_affine=True, with_bias=True,
               repeat=1, hw_loop=0, probe="full"):
    """Time the sharded executable with device-resident inputs (seconds)."""
    import time
    import jax
    from jax.sharding import NamedSharding, PartitionSpec

    _get_exec(ln_affine=ln_affine, with_bias=with_bias, repeat=repeat,
              hw_loop=hw_loop, probe=probe)
    parts = _CACHE[("parts", ln_affine, with_bias, repeat, hw_loop, probe)]
    sharded = parts["sharded"]
    n_cores = parts["n_cores"]
    in_dtypes = parts["in_dtypes"]
    sh = NamedSharding(parts["mesh"], PartitionSpec("core"))
    per_core = [[np.ascontiguousarray(np.asarray(m[name], in_dtypes[name]))
                 for name in parts["in_names"]] for m in in_maps]
    concat_in = [np.concatenate([per_core[cc][i] for cc in range(n_cores)],
                                axis=0) for i in range(parts["n_params"])]
    in_dev = [jax.device_put(a, sh) for a in concat_in]
    jax.block_until_ready(in_dev)
    times = []
    for _ in range(iters):
        z_dev = [jax.device_put(
                     np.zeros((n_cores * s[0], *s[1:]), d), sh)
                 for (s, d) in parts["zero_shapes"]]
        jax.block_until_ready(z_dev)
        t0 = time.perf_counter()
        out = sharded(*in_dev, *z_dev)
        jax.block_until_ready(out)
        times.append(time.perf_counter() - t0)
        del out
    return times


def _ln_is_identity(inputs):
    return all(
        np.all(np.asarray(inputs[k], np.float32) == v)
        for k, v in (("gq", 1.0), ("betq", 0.0), ("gkv", 1.0), ("betkv", 0.0))
    )


def _bias_is_zero(inputs):
    return all(
        np.all(np.asarray(inputs[k], np.float32) == 0.0)
        for k in ("bq", "bk", "bv")
    )


def _make_in_maps(inputs):
    import ml_dtypes
    BF = ml_dtypes.bfloat16
    q = np.asarray(inputs["query"], np.float32)
    c = np.asarray(inputs["context"], np.float32)
    Wq = np.asarray(inputs["Wq"], np.float32).astype(BF)
    Wk = np.asarray(inputs["Wk"], np.float32).astype(BF)
    Wv = np.asarray(inputs["Wv"], np.float32).astype(BF)
    Wo = np.asarray(inputs["Wo"], np.float32).astype(BF)
    bq = np.asarray(inputs["bq"], np.float32)
    bk = np.asarray(inputs["bk"], np.float32)
    bv = np.asarray(inputs["bv"], np.float32)
    gq = np.asarray(inputs["gq"], np.float32)
    btq = np.asarray(inputs["betq"], np.float32)
    gkv = np.asarray(inputs["gkv"], np.float32)
    btkv = np.asarray(inputs["betkv"], np.float32)
    ln_affine = not _ln_is_identity(inputs)
    with_bias = not _bias_is_zero(inputs)
    in_maps = []
    for core in range(8):
        b, hg = core // 2, core % 2
        sl = slice(hg * DG, (hg + 1) * DG)
        m = {
            "q_in": q[b], "c_in": c[b],
            "wq": Wq[:, sl], "wk": Wk[:, sl], "wv": Wv[:, sl],
            "wo": Wo[sl, :],
        }
        if with_bias:
            m.update({"bq": bq[sl], "bk": bk[sl], "bv": bv[sl]})
        if ln_affine:
            m.update({"gq": gq, "btq": btq, "gkv": gkv, "btkv": btkv})
        in_maps.append(m)
    return in_maps


def kernel(**inputs):
    ln_affine = not _ln_is_identity(inputs)
    with_bias = not _bias_is_zero(inputs)
    execute = _get_exec(ln_affine=ln_affine, with_bias=with_bias)
    in_maps = _make_in_maps(inputs)
    results = execute(in_maps)
    bo = np.asarray(inputs["bo"], np.float32)
    B = 4
    out = np.empty((B, N_TOK, D), np.float32)
    for b in range(B):
        out[b] = results[2 * b]["y_out"] + results[2 * b + 1]["y_out"] + bo
    return out
